# revision 20
# baseline (speedup 1.0000x reference)
"""Trainium2 Bass kernel for batched 2D variable-coefficient diffusion CG solve.

Problem: 64 independent solves of A(alpha) u = f_rhs on a 256x256 grid,
5-point stencil with edge coefficients exp(0.5*(alpha_a + alpha_b)), solved
with 300 fp32 CG iterations (the reference's jax CG never converges before
maxiter=300 at tol=1e-6 in fp32, so the output is exactly the 300th iterate).

Sharding: pure data parallel, 8 problems per NeuronCore across 8 cores.

Per-core layout: partition P = b*16 + kb (b = local problem 0..7, kb = k-block
0..15); each partition holds 16 k-columns x 256 j-rows, free index = c*256 + j
for k = kb*16 + c. All CG state lives in SBUF for all 300 iterations; the only
cross-partition traffic is a 1-column halo per side for the k-direction stencil
shifts, done with two tiny TensorE shift matmuls per iteration.

Engine split (custom fused DVE ops crash under this runtime; native ops
only): DVE does the j-direction stencil products + all sums, plus one fused
native scalar_tensor_tensor (STT) pass each for <p,Ap> (product+accumulate),
the r axpy, ||r||^2 (product+accumulate), and the p axpy; GpSimd runs the two
k-direction stencil products and the x update in parallel; ACT does the x
scale; PE does the block-diagonal ones-matmul that both segment-sums the 16
per-partition dot partials of each problem and broadcasts the result back to
its partitions, plus the two 1-column halo shift matmuls. The 300-iteration
loop is a hardware For_i unrolled x4 (back-edge barrier ~7us; unrolling also
lets the off-critical x update overlap the next iteration's stencil).

Measured on trn2 (8 cores): 76.8 us/iteration -> 23.0 ms device time for the
full solve; output matches the CPU jax reference at 1.7e-2 absmax relative
(the fp32 reproducibility envelope of this unconverged CG trajectory:
independent fp32 implementations of the same algorithm differ by ~1e-2).
Measured-and-reverted: 3 GpSimd products (84.8), GpSimd-summed k-products
(79.2), unroll8 (~79), q-recurrence variant (146, false-dep serialization),
ACT-accumulate pAp (84.8 bundled). Timing method: iteration-count slope
(300 vs 2400) isolates NEFF execution from ~230ms fixed axon dispatch.
"""

import os
import numpy as np

M = 256
B = 64
NCORES = 8
BPC = B // NCORES          # problems per core
HINV2 = np.float32(M * M)  # exact power of two: folding into coeffs is exact
ITERS = 300
COLS = 16                  # k-columns per partition
F = COLS * M               # 4096 free elements per field
FH = F + 2 * M             # p buffer with halo columns

_CACHE = {}


# ----------------------------------------------------------------- host side

def _coeff_arrays(alpha):
    """Per-problem stencil coefficient fields, matching reference._stencil_coeffs
    fp32 op-for-op, with HINV2 folded in (exact) and off-diagonals negated.

    alpha: (B, 257, 257) f32. Returns diag, KL, KB as (B, 256, 256) f32 where
    KL/KB are the *unmasked-left* / *masked-bottom* edge coefficients."""
    a = alpha.astype(np.float32)
    m = M
    j = np.arange(m)[:, None]
    k = np.arange(m)[None, :]
    KL = np.exp(np.float32(0.5) * (a[:, :-1, :-1] + a[:, :-1, 1:])).astype(np.float32)
    KR = np.where(j < m - 1,
                  np.exp(np.float32(0.5) * (a[:, 1:, :-1] + a[:, 1:, 1:])),
                  np.float32(0.0)).astype(np.float32)
    KB = np.where(k > 0,
                  np.exp(np.float32(0.5) * (a[:, :-1, :-1] + a[:, 1:, :-1])),
                  np.float32(0.0)).astype(np.float32)
    KT = np.where(k < m - 1,
                  np.exp(np.float32(0.5) * (a[:, :-1, 1:] + a[:, 1:, 1:])),
                  np.float32(0.0)).astype(np.float32)
    diag = KL + KR + KB + KT + np.where(j == 0, KL, np.float32(0.0)).astype(np.float32)
    return diag, KL, KB


def _to_dev(arr_bjk):
    """(BPC, 256j, 256k) -> [128, 4096] with P = b*16+kb, free = c*256+j."""
    t = arr_bjk.transpose(0, 2, 1)                 # (b, k, j)
    t = t.reshape(BPC, 16, COLS, M)                # (b, kb, c, j)
    return np.ascontiguousarray(t.reshape(128, F))


def _from_dev(dev):
    """[128, 4096] -> (BPC, 256j, 256k)."""
    t = dev.reshape(BPC, 16, COLS, M).transpose(0, 3, 1, 2)   # (b, j, kb, c)
    return np.ascontiguousarray(t.reshape(BPC, M, M))


def _from_dev_pad(dev):
    """[128, COLS*(M+2)] padded -> (BPC, 256j, 256k)."""
    t = dev.reshape(BPC, 16, COLS, M + 2)[:, :, :, :M].transpose(0, 3, 1, 2)
    return np.ascontiguousarray(t.reshape(BPC, M, M))


def _pack_core(alpha_core, f_rhs):
    """Build the per-core input map (all fp32 numpy arrays)."""
    diag, KL, KB = _coeff_arrays(alpha_core)
    s = HINV2
    cD = _to_dev(diag * s)                               # [128, 4096]
    nKL = _to_dev(KL * (-s)).reshape(128, COLS, M)       # (P, c, j)
    nKB = _to_dev(KB * (-s)).reshape(128, COLS, M)

    # cLp[P, c, 0..256]: 0 at jj=0 (Dirichlet kill for the j-1 shift),
    # -s*KL[jj,k] at jj=1..255, 0 at jj=256 (K_right mask at j=255).
    cLp = np.zeros((128, COLS, M + 1), np.float32)
    cLp[:, :, 1:M] = nKL[:, :, 1:M]

    # v2: contiguous copies of the two strided j-coefficient views
    cJM = np.ascontiguousarray(cLp[:, :, 0:M].reshape(128, F))      # multiplies p_jm1
    cJP = np.ascontiguousarray(cLp[:, :, 1:M + 1].reshape(128, F))  # multiplies p_jp1

    # cBp[P, 0..16, j]: c=0..15 the (already k-masked) bottom coefficients,
    # c=16 the next partition's c=0 column (static k-halo; 0 past k=255).
    cBp = np.zeros((128, COLS + 1, M), np.float32)
    cBp[:, :COLS, :] = nKB
    nKB4 = nKB.reshape(BPC, 16, COLS, M)
    cBp4 = cBp.reshape(BPC, 16, COLS + 1, M)
    cBp4[:, :-1, COLS, :] = nKB4[:, 1:, 0, :]

    fdev = _to_dev(np.broadcast_to(f_rhs, (BPC, M, M)).astype(np.float32))

    # ---- v3 flux-form packing: padded columns (S=258 slots, 2 zero pads)
    S = M + 2
    FP = COLS * S
    def _pad_cols(flat128):                       # [128, F] -> [128, FP]
        t = flat128.reshape(128, COLS, M)
        out = np.zeros((128, COLS, S), np.float32)
        out[:, :, :M] = t
        return out.reshape(128, FP)
    fpad = _pad_cols(fdev)
    xKL = _to_dev((KL * s).astype(np.float32))    # +s*KL unmasked, dev layout
    cJF = _pad_cols(xKL)
    cJF3 = cJF.reshape(128, COLS, S)
    cJF3[:, :, 0] = (np.float32(2.0) * xKL.reshape(128, COLS, M)[:, :, 0])
    # j-faces 256..257 are pad faces = 0; face 256 would be s*KR[255] = 0 anyway
    cJF = np.ascontiguousarray(cJF3.reshape(128, FP))
    # k-faces: 17 per partition, +s*K = -cBp (cBp holds -s*K, k-masked, with halo col)
    cKF = np.zeros((128, COLS + 1, S), np.float32)
    cKF[:, :, :M] = -cBp
    cKF = np.ascontiguousarray(cKF.reshape(128, (COLS + 1) * S))

    seg = np.zeros((128, BPC), np.float32)               # seg[q, b] = q//16 == b
    seg[np.arange(128), np.arange(128) // 16] = 1.0
    bc = np.ascontiguousarray(seg.T)                     # (8, 128)
    qi = np.arange(128)
    bc128 = (qi[:, None] // 16 == qi[None, :] // 16).astype(np.float32)
    sdn = np.eye(128, 128, 1, np.float32)                # out[i] = in[i-1]
    sup = np.eye(128, 128, -1, np.float32)               # out[i] = in[i+1]

    return {
        "fp_in": fpad,
        "cJF_in": cJF,
        "cKF_in": cKF,
        "f_in": fdev,
        "cD_in": cD,
        "cL_in": np.ascontiguousarray(cLp.reshape(128, COLS * (M + 1))),
        "cJM_in": cJM,
        "cJP_in": cJP,
        "cB_in": np.ascontiguousarray(cBp.reshape(128, (COLS + 1) * M)),
        "seg_in": seg,
        "bc_in": bc,
        "bc128_in": bc128,
        "sdn_in": sdn,
        "sup_in": sup,
    }


# --------------------------------------------------------------- bass kernel

def _build_nc_qrec(iters):
    """q-recurrence variant: q_{k+1} = A r_{k+1} + beta_k q_k.

    The stencil runs on r (available right after the r update), so the
    ||r||^2 / beta / p-update chain hides behind it. Validated in exp3.py:
    lands as close to the f64 trajectory as plain fp32 CG.

    Loop state: p, q (= A p), r (halo'd), x, gamvec ([128,1] per-problem
    gamma broadcast). Body:
        pAp = <p, q>; alpha = gamma/pAp
        x += alpha p ; r -= alpha q ; refresh r halos
        gamma' = ||r||^2 ; beta = gamma'/gamma
        w = A r  (overlaps beta chain and p update)
        p = r + beta p ; q = w + beta q
    """
    from contextlib import ExitStack
    import concourse.bass as bass
    import concourse.tile as tile
    from concourse import bacc, mybir

    f32 = mybir.dt.float32
    Alu = mybir.AluOpType
    Act = mybir.ActivationFunctionType

    nc = bacc.Bacc("TRN2", target_bir_lowering=False, debug=False)

    f_d = nc.dram_tensor("f_in", [128, F], f32, kind="ExternalInput").ap()
    cD_d = nc.dram_tensor("cD_in", [128, F], f32, kind="ExternalInput").ap()
    cL_d = nc.dram_tensor("cL_in", [128, COLS * (M + 1)], f32, kind="ExternalInput").ap()
    cB_d = nc.dram_tensor("cB_in", [128, (COLS + 1) * M], f32, kind="ExternalInput").ap()
    bc128_d = nc.dram_tensor("bc128_in", [128, 128], f32, kind="ExternalInput").ap()
    sdn_d = nc.dram_tensor("sdn_in", [128, 128], f32, kind="ExternalInput").ap()
    sup_d = nc.dram_tensor("sup_in", [128, 128], f32, kind="ExternalInput").ap()
    x_d = nc.dram_tensor("x_out", [128, F], f32, kind="ExternalOutput").ap()

    with tile.TileContext(nc) as tc, ExitStack() as ctx:
        sb = ctx.enter_context(tc.tile_pool(name="state", bufs=1))
        ps = ctx.enter_context(tc.tile_pool(name="psum", bufs=1, space="PSUM"))

        r = sb.tile([128, FH], f32)       # halo_lo | center | halo_hi
        p = sb.tile([128, F], f32)
        x = sb.tile([128, F], f32)
        q = sb.tile([128, F], f32)        # A @ p via recurrence
        t0 = sb.tile([128, F], f32)
        t1 = sb.tile([128, F], f32)
        t2 = sb.tile([128, F], f32)
        t3 = sb.tile([128, F], f32)
        cD = sb.tile([128, F], f32)
        cL = sb.tile([128, COLS * (M + 1)], f32)
        cB = sb.tile([128, (COLS + 1) * M], f32)
        bc128 = sb.tile([128, 128], f32)
        sdn = sb.tile([128, 128], f32)
        sup = sb.tile([128, 128], f32)

        pap_part = sb.tile([128, 1], f32)
        rr_part = sb.tile([128, 1], f32)
        gamvec = sb.tile([128, 1], f32)   # per-problem gamma, broadcast
        recg = sb.tile([128, 1], f32)     # 1/gamma_old
        recp = sb.tile([128, 1], f32)     # 1/pAp
        avec = sb.tile([128, 1], f32)
        bvec = sb.tile([128, 1], f32)

        pap_ps = ps.tile([128, 1], f32)
        gam_ps = ps.tile([128, 1], f32)
        hlo_ps = ps.tile([128, M], f32)
        hhi_ps = ps.tile([128, M], f32)

        def v3(ap2d):
            return ap2d.rearrange("p (c j) -> p c j", c=COLS, j=M)

        r_c2 = r[:, M:M + F]
        r_c3 = v3(r_c2)
        r_jm1 = v3(r[:, M - 1:M - 1 + F])
        r_jp1 = v3(r[:, M + 1:M + 1 + F])
        r_km1 = v3(r[:, 0:F])
        r_kp1 = v3(r[:, 2 * M:2 * M + F])
        cL3 = cL[:].rearrange("p (c j) -> p c j", c=COLS, j=M + 1)
        cLl = cL3[:, :, 0:M]
        cLr = cL3[:, :, 1:M + 1]
        cB3 = cB[:].rearrange("p (c j) -> p c j", c=COLS + 1, j=M)
        cBb = cB3[:, 0:COLS, :]
        cBt = cB3[:, 1:COLS + 1, :]
        cD3 = v3(cD[:])

        nc.sync.dma_start(cD[:], cD_d)
        nc.sync.dma_start(cL[:], cL_d)
        nc.sync.dma_start(cB[:], cB_d)
        nc.sync.dma_start(bc128[:], bc128_d)
        nc.sync.dma_start(sdn[:], sdn_d)
        nc.sync.dma_start(sup[:], sup_d)
        nc.sync.dma_start(r_c2, f_d)
        nc.sync.dma_start(p[:], f_d)

        def halo_update():
            nc.tensor.matmul(hlo_ps[:], sdn[:], r[:, F:F + M])
            nc.tensor.matmul(hhi_ps[:], sup[:], r[:, M:2 * M])
            nc.scalar.copy(r[:, 0:M], hlo_ps[:])
            nc.scalar.copy(r[:, F + M:F + 2 * M], hhi_ps[:])

        def stencil_w():
            """t0 = A @ r (j-terms on DVE, k-products on GpSimd)."""
            nc.gpsimd.tensor_mul(v3(t2[:]), cBb, r_km1)
            nc.gpsimd.tensor_mul(v3(t3[:]), cBt, r_kp1)
            nc.vector.tensor_mul(v3(t0[:]), cD3, r_c3)
            nc.vector.tensor_mul(v3(t1[:]), cLl, r_jm1)
            nc.vector.tensor_add(t0[:], t0[:], t1[:])
            nc.vector.tensor_mul(v3(t1[:]), cLr, r_jp1)
            nc.vector.tensor_add(t0[:], t0[:], t1[:])
            nc.vector.tensor_add(t0[:], t0[:], t2[:])
            nc.vector.tensor_add(t0[:], t0[:], t3[:])

        # ---- init: x=0, r=p=f, q = A p, gamma0
        nc.vector.memset(x[:], 0.0)
        halo_update()
        nc.scalar.activation(t1[:], r_c2, Act.Square, accum_out=rr_part[:])
        nc.tensor.matmul(gam_ps[:], bc128[:], rr_part[:])
        nc.scalar.copy(gamvec[:], gam_ps[:])
        stencil_w()
        nc.vector.tensor_copy(q[:], t0[:])

        # ---- 300 CG iterations
        with tc.For_i(0, iters) as _i:
            nc.vector.reciprocal(recg[:], gamvec[:])

            # pAp and alpha
            nc.vector.tensor_mul(t3[:], p[:], q[:])
            nc.scalar.activation(t3[:], t3[:], Act.Copy, accum_out=pap_part[:])
            nc.tensor.matmul(pap_ps[:], bc128[:], pap_part[:])
            nc.vector.reciprocal(recp[:], pap_ps[:])
            nc.vector.tensor_mul(avec[:], gamvec[:], recp[:])

            # x += alpha*p (ACT+GpSimd, off critical) ; r -= alpha*q (DVE)
            nc.scalar.activation(t2[:], p[:], Act.Copy, scale=avec[:])
            nc.gpsimd.tensor_add(x[:], x[:], t2[:])
            nc.vector.tensor_scalar_mul(t1[:], q[:], avec[:])
            nc.vector.tensor_sub(r_c2, r_c2, t1[:])
            halo_update()

            # gamma' and beta (hidden under the stencil)
            nc.scalar.activation(t1[:], r_c2, Act.Square, accum_out=rr_part[:])
            nc.tensor.matmul(gam_ps[:], bc128[:], rr_part[:])
            nc.vector.tensor_mul(bvec[:], gam_ps[:], recg[:])
            nc.scalar.copy(gamvec[:], gam_ps[:])

            # w = A r
            stencil_w()

            # p = r + beta*p (GpSimd) ; q = w + beta*q (DVE)
            nc.gpsimd.tensor_scalar_mul(t2[:], p[:], bvec[:])
            nc.gpsimd.tensor_add(p[:], r_c2, t2[:])
            nc.vector.tensor_scalar_mul(t1[:], q[:], bvec[:])
            nc.vector.tensor_add(q[:], t0[:], t1[:])

        nc.sync.dma_start(x_d, x[:])

    nc.compile()
    return nc


def _build_nc_v2(iters):
    """Bit-exact restructuring of the std kernel:

    - strided cLl/cLr coefficient views replaced by contiguous cJM/cJP arrays
    - every field op is a flat 2D contiguous window (no 3D APs on DVE)
    - alpha chain tightened: aneg = (-gamma)*recp directly (one op after PE)
    - optional GpSimd tail-splits on the j-stencil/add ops (SPLITS dict),
      bit-exact since IEEE elementwise ops match across engines
    """
    from contextlib import ExitStack
    import concourse.bass as bass
    import concourse.tile as tile
    from concourse import bacc, mybir
    import json

    f32 = mybir.dt.float32
    Alu = mybir.AluOpType
    Act = mybir.ActivationFunctionType

    # gp tail elements per op site (0 = whole op on DVE)
    SP = {"mcd": 0, "mjm": 0, "a1": 0, "mjp": 0, "a2": 0, "a3": 0, "a4": 0,
          "rup": 0, "pup": 0}
    env = os.environ.get("KERNEL_SPLITS")
    if env:
        SP.update(json.loads(env))
    # engine modes: kprod = dve|gp (the 2 k-shift products), xupd = stt|actgp
    KPROD = os.environ.get("KERNEL_KPROD", "dve")
    XUPD = os.environ.get("KERNEL_XUPD", "stt")
    RRENG = os.environ.get("KERNEL_RR", "dve")  # dve|act (act changes accum order!)

    nc = bacc.Bacc("TRN2", target_bir_lowering=False, debug=False)

    f_d = nc.dram_tensor("f_in", [128, F], f32, kind="ExternalInput").ap()
    cD_d = nc.dram_tensor("cD_in", [128, F], f32, kind="ExternalInput").ap()
    cJM_d = nc.dram_tensor("cJM_in", [128, F], f32, kind="ExternalInput").ap()
    cJP_d = nc.dram_tensor("cJP_in", [128, F], f32, kind="ExternalInput").ap()
    cB_d = nc.dram_tensor("cB_in", [128, (COLS + 1) * M], f32, kind="ExternalInput").ap()
    bc128_d = nc.dram_tensor("bc128_in", [128, 128], f32, kind="ExternalInput").ap()
    sdn_d = nc.dram_tensor("sdn_in", [128, 128], f32, kind="ExternalInput").ap()
    sup_d = nc.dram_tensor("sup_in", [128, 128], f32, kind="ExternalInput").ap()
    x_d = nc.dram_tensor("x_out", [128, F], f32, kind="ExternalOutput").ap()

    with tile.TileContext(nc) as tc, ExitStack() as ctx:
        sb = ctx.enter_context(tc.tile_pool(name="state", bufs=1))
        ps = ctx.enter_context(tc.tile_pool(name="psum", bufs=1, space="PSUM"))

        p = sb.tile([128, FH], f32)       # halo_lo | center | halo_hi
        r = sb.tile([128, F], f32)
        x = sb.tile([128, F], f32)
        q = sb.tile([128, F], f32)
        t0 = sb.tile([128, F], f32)
        t1 = sb.tile([128, F], f32)
        t2 = sb.tile([128, F], f32)
        t3 = sb.tile([128, F], f32)
        cD = sb.tile([128, F], f32)
        cJM = sb.tile([128, F], f32)
        cJP = sb.tile([128, F], f32)
        cB = sb.tile([128, (COLS + 1) * M], f32)
        bc128 = sb.tile([128, 128], f32)
        sdn = sb.tile([128, 128], f32)
        sup = sb.tile([128, 128], f32)

        pap_part = sb.tile([128, 1], f32)
        rr_part = sb.tile([128, 1], f32)
        gamvec = sb.tile([128, 1], f32)
        gneg = sb.tile([128, 1], f32)     # -gamma (for aneg, off critical)
        recg = sb.tile([128, 1], f32)
        recp = sb.tile([128, 1], f32)
        avec = sb.tile([128, 1], f32)
        aneg = sb.tile([128, 1], f32)
        bvec = sb.tile([128, 1], f32)

        pap_ps = ps.tile([128, 1], f32)
        gam_ps = ps.tile([128, 1], f32)
        hlo_ps = ps.tile([128, M], f32)
        hhi_ps = ps.tile([128, M], f32)

        # flat contiguous windows
        pc = p[:, M:M + F]
        pjm = p[:, M - 1:M - 1 + F]
        pjp = p[:, M + 1:M + 1 + F]
        pkm = p[:, 0:F]
        pkp = p[:, 2 * M:2 * M + F]
        cBb = cB[:, 0:F]
        cBt = cB[:, M:M + F]

        # ---- load inputs
        nc.sync.dma_start(cD[:], cD_d)
        nc.sync.dma_start(cJM[:], cJM_d)
        nc.sync.dma_start(cJP[:], cJP_d)
        nc.sync.dma_start(cB[:], cB_d)
        nc.sync.dma_start(bc128[:], bc128_d)
        nc.sync.dma_start(sdn[:], sdn_d)
        nc.sync.dma_start(sup[:], sup_d)
        nc.sync.dma_start(r[:], f_d)
        nc.sync.dma_start(pc, f_d)

        def halo_update():
            nc.tensor.matmul(hlo_ps[:], sdn[:], p[:, F:F + M])
            nc.tensor.matmul(hhi_ps[:], sup[:], p[:, M:2 * M])
            nc.scalar.copy(p[:, 0:M], hlo_ps[:])
            nc.scalar.copy(p[:, F + M:F + 2 * M], hhi_ps[:])

        def mul2(site, out, in0, in1):
            """TT mul with optional GpSimd tail split (bit-exact)."""
            s = SP[site]
            if s:
                nc.vector.tensor_mul(out[:, :F - s], in0[:, :F - s], in1[:, :F - s])
                nc.gpsimd.tensor_mul(out[:, F - s:], in0[:, F - s:], in1[:, F - s:])
            else:
                nc.vector.tensor_mul(out, in0, in1)

        def add2(site, out, in0, in1):
            s = SP[site]
            if s:
                nc.vector.tensor_add(out[:, :F - s], in0[:, :F - s], in1[:, :F - s])
                nc.gpsimd.tensor_add(out[:, F - s:], in0[:, F - s:], in1[:, F - s:])
            else:
                nc.vector.tensor_add(out, in0, in1)

        # ---- init: x=0, gamma0 = per-problem ||f||^2, p halos
        nc.vector.memset(x[:], 0.0)
        halo_update()
        nc.scalar.activation(t1[:], r[:], Act.Square, accum_out=rr_part[:])
        nc.tensor.matmul(gam_ps[:], bc128[:], rr_part[:])
        nc.scalar.copy(gamvec[:], gam_ps[:])
        nc.vector.tensor_scalar_mul(gneg[:], gamvec[:], -1.0)

        # ---- 300 CG iterations
        def body(_i):
            nc.vector.reciprocal(recg[:], gamvec[:])

            # q = A @ p
            if KPROD == "gp":
                nc.gpsimd.tensor_mul(t2[:], cBb, pkm)
                nc.gpsimd.tensor_mul(t3[:], cBt, pkp)
            else:
                nc.vector.tensor_mul(t2[:], cBb, pkm)
                nc.vector.tensor_mul(t3[:], cBt, pkp)
            mul2("mcd", t0, cD[:], pc)
            mul2("mjm", t1, cJM[:], pjm)
            add2("a1", t0, t0[:], t1[:])
            mul2("mjp", t1, cJP[:], pjp)
            add2("a2", t0, t0[:], t1[:])
            add2("a3", t0, t0[:], t2[:])
            add2("a4", q, t0[:], t3[:])

            # pAp = sum(p*q) fused in one DVE pass; alpha = gamma/pAp
            nc.vector.scalar_tensor_tensor(
                t3[:], pc, 1.0, q[:], Alu.mult, Alu.mult,
                accum_out=pap_part[:])
            nc.tensor.matmul(pap_ps[:], bc128[:], pap_part[:])
            nc.vector.reciprocal(recp[:], pap_ps[:])
            # aneg = (-gamma) * recp  == -(gamma*recp) bit-exactly
            nc.vector.tensor_mul(aneg[:], gneg[:], recp[:])

            # r = (q * -alpha) + r, one pass
            s = SP["rup"]
            if s:
                nc.vector.scalar_tensor_tensor(
                    r[:, :F - s], q[:, :F - s], aneg[:], r[:, :F - s],
                    Alu.mult, Alu.add)
                nc.gpsimd.tensor_scalar_mul(t0[:, F - s:], q[:, F - s:], aneg[:])
                nc.gpsimd.tensor_add(r[:, F - s:], t0[:, F - s:], r[:, F - s:])
            else:
                nc.vector.scalar_tensor_tensor(
                    r[:], q[:], aneg[:], r[:], Alu.mult, Alu.add)

            # x += alpha*p off-critical
            nc.vector.tensor_mul(avec[:], gamvec[:], recp[:])
            if XUPD == "actgp":
                nc.scalar.activation(t1[:], pc, Act.Copy, scale=avec[:])
                nc.gpsimd.tensor_add(x[:], x[:], t1[:])
            else:  # single DVE STT: x = (p*alpha) + x, bit-exact same values
                nc.vector.scalar_tensor_tensor(
                    x[:], pc, avec[:], x[:], Alu.mult, Alu.add)

            # gamma' = sum(r*r); beta
            if RRENG == "act":
                nc.scalar.activation(t2[:], r[:], Act.Square, accum_out=rr_part[:])
            else:
                nc.vector.scalar_tensor_tensor(
                    t2[:], r[:], 1.0, r[:], Alu.mult, Alu.mult,
                    accum_out=rr_part[:])
            nc.tensor.matmul(gam_ps[:], bc128[:], rr_part[:])
            nc.vector.tensor_mul(bvec[:], gam_ps[:], recg[:])
            nc.scalar.copy(gamvec[:], gam_ps[:])
            nc.vector.tensor_scalar_mul(gneg[:], gamvec[:], -1.0)

            # p = (p * beta) + r in one pass, then refresh halos
            s = SP["pup"]
            if s:
                nc.vector.scalar_tensor_tensor(
                    pc[:, :F - s], pc[:, :F - s], bvec[:], r[:, :F - s],
                    Alu.mult, Alu.add)
                nc.gpsimd.tensor_scalar_mul(t2[:, F - s:], pc[:, F - s:], bvec[:])
                nc.gpsimd.tensor_add(pc[:, F - s:], t2[:, F - s:], r[:, F - s:])
            else:
                nc.vector.scalar_tensor_tensor(
                    pc, pc, bvec[:], r[:], Alu.mult, Alu.add)
            halo_update()

        loop_mode = os.environ.get("KERNEL_LOOP", "unroll4")
        if loop_mode == "plain":
            with tc.For_i(0, iters) as _i:
                body(_i)
        elif loop_mode == "stag":
            with tc.For_i(0, iters, staggered_reset=True) as _i:
                body(_i)
        elif loop_mode.startswith("unroll"):
            tc.For_i_unrolled(0, iters, 1, body, max_unroll=int(loop_mode[6:]))
        else:
            raise ValueError(loop_mode)

        nc.sync.dma_start(x_d, x[:])

    nc.compile()
    return nc


def _build_nc_v3(iters):
    """Flux-form stencil in a padded-column layout: 7 stencil passes vs 9.

    Each column holds S=258 slots (256 rows + 2 zero pads). The pads supply
    the Dirichlet/Neumann zeros so every stencil op is a flat contiguous
    window subtract/multiply:
        dj = p - p(<<1); Fj = cJF*dj; qj = Fj - Fj(>>1)
        dk = p - p(<<S); Fk = cKF*dk; q = (qj + Fk_lo) - Fk_hi
    Dot products run flat over the padded range (pad terms are exact +0.0,
    so the accumulation stream matches the unpadded order bit-for-bit);
    r/p updates use pad-skipping 3D views so the pads stay exactly zero.
    Trajectory note: the flux association differs from the operator form;
    measured on CPU at 1.66e-2 vs reference (same class as op form).
    """
    from contextlib import ExitStack
    import concourse.bass as bass
    import concourse.tile as tile
    from concourse import bacc, mybir

    f32 = mybir.dt.float32
    Alu = mybir.AluOpType
    Act = mybir.ActivationFunctionType

    S = M + 2
    FP = COLS * S
    FHP = FP + 2 * S
    KFP = (COLS + 1) * S
    RRENG = os.environ.get("KERNEL_RR", "act")

    nc = bacc.Bacc("TRN2", target_bir_lowering=False, debug=False)

    fp_d = nc.dram_tensor("fp_in", [128, FP], f32, kind="ExternalInput").ap()
    cJF_d = nc.dram_tensor("cJF_in", [128, FP], f32, kind="ExternalInput").ap()
    cKF_d = nc.dram_tensor("cKF_in", [128, KFP], f32, kind="ExternalInput").ap()
    bc128_d = nc.dram_tensor("bc128_in", [128, 128], f32, kind="ExternalInput").ap()
    sdn_d = nc.dram_tensor("sdn_in", [128, 128], f32, kind="ExternalInput").ap()
    sup_d = nc.dram_tensor("sup_in", [128, 128], f32, kind="ExternalInput").ap()
    x_d = nc.dram_tensor("xp_out", [128, FP], f32, kind="ExternalOutput").ap()

    with tile.TileContext(nc) as tc, ExitStack() as ctx:
        sb = ctx.enter_context(tc.tile_pool(name="state", bufs=1))
        ps = ctx.enter_context(tc.tile_pool(name="psum", bufs=1, space="PSUM"))

        p = sb.tile([128, FHP], f32)     # halo_lo(S) | center(FP) | halo_hi(S)
        r = sb.tile([128, FP], f32)
        x = sb.tile([128, FP], f32)
        q = sb.tile([128, FP], f32)
        t0 = sb.tile([128, FP], f32)
        t1 = sb.tile([128, FP + 8], f32)
        t2 = sb.tile([128, KFP], f32)
        cJF = sb.tile([128, FP], f32)
        cKF = sb.tile([128, KFP], f32)
        bc128 = sb.tile([128, 128], f32)
        sdn = sb.tile([128, 128], f32)
        sup = sb.tile([128, 128], f32)

        pap_part = sb.tile([128, 1], f32)
        rr_part = sb.tile([128, 1], f32)
        gamvec = sb.tile([128, 1], f32)
        gneg = sb.tile([128, 1], f32)
        recg = sb.tile([128, 1], f32)
        recp = sb.tile([128, 1], f32)
        avec = sb.tile([128, 1], f32)
        aneg = sb.tile([128, 1], f32)
        bvec = sb.tile([128, 1], f32)

        pap_ps = ps.tile([128, 1], f32)
        gam_ps = ps.tile([128, 1], f32)
        hlo_ps = ps.tile([128, S], f32)
        hhi_ps = ps.tile([128, S], f32)

        pcen = p[:, S:S + FP]

        def vv(ap2d):
            """pad-skipping 3D view over a [128, FP] range."""
            return ap2d.rearrange("p (c j) -> p c j", c=COLS, j=S)[:, :, 0:M]

        # ---- load inputs / init
        nc.sync.dma_start(cJF[:], cJF_d)
        nc.sync.dma_start(cKF[:], cKF_d)
        nc.sync.dma_start(bc128[:], bc128_d)
        nc.sync.dma_start(sdn[:], sdn_d)
        nc.sync.dma_start(sup[:], sup_d)
        nc.vector.memset(p[:], 0.0)
        nc.vector.memset(x[:], 0.0)
        nc.vector.memset(t1[:], 0.0)
        nc.sync.dma_start(r[:], fp_d)
        nc.sync.dma_start(pcen, fp_d)

        def halo_update():
            # copy only the 256 real slots: pads stay 0 from init, and the
            # j-stencil's read of the always-zero pad slot S-1 no longer
            # falsely depends on this round trip
            nc.tensor.matmul(hlo_ps[:, 0:M], sdn[:], p[:, FP:FP + M])
            nc.tensor.matmul(hhi_ps[:, 0:M], sup[:], p[:, S:S + M])
            nc.scalar.copy(p[:, 0:M], hlo_ps[:, 0:M])
            nc.scalar.copy(p[:, S + FP:S + FP + M], hhi_ps[:, 0:M])

        halo_update()
        nc.scalar.activation(t0[:], r[:], Act.Square, accum_out=rr_part[:])
        nc.tensor.matmul(gam_ps[:], bc128[:], rr_part[:])
        nc.scalar.copy(gamvec[:], gam_ps[:])
        nc.vector.tensor_scalar_mul(gneg[:], gamvec[:], -1.0)

        def body(_i):
            nc.vector.reciprocal(recg[:], gamvec[:])

            # q = A p, flux form: 7 flat passes
            nc.vector.tensor_sub(t1[:, 0:FP], pcen, p[:, S - 1:S - 1 + FP])
            nc.vector.tensor_mul(t1[:, 0:FP], cJF[:], t1[:, 0:FP])
            nc.vector.tensor_sub(t0[:], t1[:, 0:FP], t1[:, 1:FP + 1])
            nc.vector.tensor_sub(t2[:], p[:, S:S + KFP], p[:, 0:KFP])
            nc.vector.tensor_mul(t2[:], cKF[:], t2[:])
            nc.vector.tensor_add(q[:], t0[:], t2[:, 0:FP])
            nc.vector.tensor_sub(q[:], q[:], t2[:, S:S + FP])

            # pAp (flat; pad terms are exact zeros) ; alpha = gamma/pAp
            nc.vector.scalar_tensor_tensor(
                t0[:], pcen, 1.0, q[:], Alu.mult, Alu.mult,
                accum_out=pap_part[:])
            nc.tensor.matmul(pap_ps[:], bc128[:], pap_part[:])
            nc.vector.reciprocal(recp[:], pap_ps[:])
            nc.vector.tensor_mul(aneg[:], gneg[:], recp[:])

            # r = (q * -alpha) + r  (pad-skipping: r pads stay 0)
            nc.vector.scalar_tensor_tensor(
                vv(r[:]), vv(q[:]), aneg[:], vv(r[:]), Alu.mult, Alu.add)

            # gamma' = ||r||^2 immediately after r (flat; pads exact zeros)
            if RRENG == "act":
                nc.scalar.activation(t2[:, 0:FP], r[:], Act.Square,
                                     accum_out=rr_part[:])
            else:
                nc.vector.scalar_tensor_tensor(
                    t2[:, 0:FP], r[:], 1.0, r[:], Alu.mult, Alu.mult,
                    accum_out=rr_part[:])
            nc.tensor.matmul(gam_ps[:], bc128[:], rr_part[:])

            # x += alpha*p fills the gamma PE-trip window (flat; pads stay 0)
            nc.vector.tensor_mul(avec[:], gamvec[:], recp[:])
            nc.vector.scalar_tensor_tensor(
                x[:], pcen, avec[:], x[:], Alu.mult, Alu.add)

            nc.vector.tensor_mul(bvec[:], gam_ps[:], recg[:])
            nc.scalar.copy(gamvec[:], gam_ps[:])
            nc.vector.tensor_scalar_mul(gneg[:], gamvec[:], -1.0)

            # p = (p * beta) + r (pad-skipping: p pads stay 0), then halos
            nc.vector.scalar_tensor_tensor(
                vv(pcen), vv(pcen), bvec[:], vv(r[:]), Alu.mult, Alu.add)
            halo_update()

        loop_mode = os.environ.get("KERNEL_LOOP", "unroll8")
        if loop_mode == "plain":
            with tc.For_i(0, iters) as _i:
                body(_i)
        elif loop_mode.startswith("unroll"):
            tc.For_i_unrolled(0, iters, 1, body, max_unroll=int(loop_mode[6:]))
        else:
            raise ValueError(loop_mode)

        nc.sync.dma_start(x_d, x[:])

    nc.compile()
    return nc


def _build_nc(iters):
    from contextlib import ExitStack
    import concourse.bass as bass
    import concourse.tile as tile
    from concourse import bacc, mybir

    f32 = mybir.dt.float32
    Alu = mybir.AluOpType
    Act = mybir.ActivationFunctionType

    nc = bacc.Bacc("TRN2", target_bir_lowering=False, debug=False)

    f_d = nc.dram_tensor("f_in", [128, F], f32, kind="ExternalInput").ap()
    cD_d = nc.dram_tensor("cD_in", [128, F], f32, kind="ExternalInput").ap()
    cL_d = nc.dram_tensor("cL_in", [128, COLS * (M + 1)], f32, kind="ExternalInput").ap()
    cB_d = nc.dram_tensor("cB_in", [128, (COLS + 1) * M], f32, kind="ExternalInput").ap()
    bc128_d = nc.dram_tensor("bc128_in", [128, 128], f32, kind="ExternalInput").ap()
    sdn_d = nc.dram_tensor("sdn_in", [128, 128], f32, kind="ExternalInput").ap()
    sup_d = nc.dram_tensor("sup_in", [128, 128], f32, kind="ExternalInput").ap()
    x_d = nc.dram_tensor("x_out", [128, F], f32, kind="ExternalOutput").ap()

    with tile.TileContext(nc) as tc, ExitStack() as ctx:
        sb = ctx.enter_context(tc.tile_pool(name="state", bufs=1))
        ps = ctx.enter_context(tc.tile_pool(name="psum", bufs=1, space="PSUM"))

        p = sb.tile([128, FH], f32)       # halo_lo | center | halo_hi
        r = sb.tile([128, F], f32)
        x = sb.tile([128, F], f32)
        q = sb.tile([128, F], f32)        # A @ p
        t0 = sb.tile([128, F], f32)       # DVE stencil accumulator
        t1 = sb.tile([128, F], f32)       # DVE-only scratch (products, axpy terms)
        t2 = sb.tile([128, F], f32)       # GpSimd m3 product / ACT rr junk
        t3 = sb.tile([128, F], f32)       # GpSimd m4 product / pAp product / x term
        t4 = sb.tile([128, F], f32)       # GpSimd m1 product (dedicated)
        cD = sb.tile([128, F], f32)
        cL = sb.tile([128, COLS * (M + 1)], f32)
        cB = sb.tile([128, (COLS + 1) * M], f32)
        bc128 = sb.tile([128, 128], f32)
        sdn = sb.tile([128, 128], f32)
        sup = sb.tile([128, 128], f32)

        pap_part = sb.tile([128, 1], f32)
        rr_part = sb.tile([128, 1], f32)
        gamvec = sb.tile([128, 1], f32)   # per-problem gamma, broadcast
        recg = sb.tile([128, 1], f32)
        recp = sb.tile([128, 1], f32)
        avec = sb.tile([128, 1], f32)
        aneg = sb.tile([128, 1], f32)
        bvec = sb.tile([128, 1], f32)

        pap_ps = ps.tile([128, 1], f32)
        gam_ps = ps.tile([128, 1], f32)
        hlo_ps = ps.tile([128, M], f32)
        hhi_ps = ps.tile([128, M], f32)

        # 3D views [128, 16, 256] over the stencil operands
        def v3(ap2d):
            return ap2d.rearrange("p (c j) -> p c j", c=COLS, j=M)

        p_c2 = p[:, M:M + F]
        p_c3 = v3(p_c2)
        p_jm1 = v3(p[:, M - 1:M - 1 + F])
        p_jp1 = v3(p[:, M + 1:M + 1 + F])
        p_km1 = v3(p[:, 0:F])
        p_kp1 = v3(p[:, 2 * M:2 * M + F])
        cL3 = cL[:].rearrange("p (c j) -> p c j", c=COLS, j=M + 1)
        cLl = cL3[:, :, 0:M]        # multiplies p_jm1
        cLr = cL3[:, :, 1:M + 1]    # multiplies p_jp1 (= K_right view)
        cB3 = cB[:].rearrange("p (c j) -> p c j", c=COLS + 1, j=M)
        cBb = cB3[:, 0:COLS, :]     # multiplies p_km1
        cBt = cB3[:, 1:COLS + 1, :] # multiplies p_kp1 (= K_top view)
        cD3 = v3(cD[:])

        # ---- load inputs
        nc.sync.dma_start(cD[:], cD_d)
        nc.sync.dma_start(cL[:], cL_d)
        nc.sync.dma_start(cB[:], cB_d)
        nc.sync.dma_start(bc128[:], bc128_d)
        nc.sync.dma_start(sdn[:], sdn_d)
        nc.sync.dma_start(sup[:], sup_d)
        nc.sync.dma_start(r[:], f_d)
        nc.sync.dma_start(p_c2, f_d)

        def halo_update():
            # halo_lo[P] = center_last_col[P-1]; halo_hi[P] = center_first_col[P+1]
            nc.tensor.matmul(hlo_ps[:], sdn[:], p[:, F:F + M])
            nc.tensor.matmul(hhi_ps[:], sup[:], p[:, M:2 * M])
            nc.scalar.copy(p[:, 0:M], hlo_ps[:])
            nc.scalar.copy(p[:, F + M:F + 2 * M], hhi_ps[:])

        # ---- init: x=0, gamma0 = per-problem ||f||^2, p halos
        nc.vector.memset(x[:], 0.0)
        halo_update()
        nc.scalar.activation(t1[:], r[:], Act.Square, accum_out=rr_part[:])
        nc.tensor.matmul(gam_ps[:], bc128[:], rr_part[:])
        nc.scalar.copy(gamvec[:], gam_ps[:])

        # ---- 300 CG iterations
        loop_mode = os.environ.get("KERNEL_LOOP", "unroll4")

        def body(_i):
            # 1/gamma_old for beta, overlappable with the stencil
            nc.vector.reciprocal(recg[:], gamvec[:])

            # q = A @ p  (GpSimd: k-shift products; DVE: the rest)
            nc.gpsimd.tensor_mul(v3(t2[:]), cBb, p_km1)
            nc.gpsimd.tensor_mul(v3(t3[:]), cBt, p_kp1)
            nc.vector.tensor_mul(v3(t0[:]), cD3, p_c3)
            nc.vector.tensor_mul(v3(t1[:]), cLl, p_jm1)
            nc.vector.tensor_add(t0[:], t0[:], t1[:])
            nc.vector.tensor_mul(v3(t1[:]), cLr, p_jp1)
            nc.vector.tensor_add(t0[:], t0[:], t1[:])
            nc.vector.tensor_add(t0[:], t0[:], t2[:])
            nc.vector.tensor_add(q[:], t0[:], t3[:])

            # pAp = sum(p*q) fused in one DVE pass; alpha = gamma/pAp
            nc.vector.scalar_tensor_tensor(
                t3[:], p_c2, 1.0, q[:], Alu.mult, Alu.mult,
                accum_out=pap_part[:])
            nc.tensor.matmul(pap_ps[:], bc128[:], pap_part[:])
            nc.vector.reciprocal(recp[:], pap_ps[:])
            nc.vector.tensor_mul(avec[:], gamvec[:], recp[:])
            nc.vector.tensor_scalar_mul(aneg[:], avec[:], -1.0)

            # r = (q * -alpha) + r, one pass; x += alpha*p off-critical
            nc.vector.scalar_tensor_tensor(
                r[:], q[:], aneg[:], r[:], Alu.mult, Alu.add)
            nc.scalar.activation(t3[:], p_c2, Act.Copy, scale=avec[:])
            nc.gpsimd.tensor_add(x[:], x[:], t3[:])

            # gamma' = sum(r*r) fused on DVE (no engine hop); beta
            nc.vector.scalar_tensor_tensor(
                t2[:], r[:], 1.0, r[:], Alu.mult, Alu.mult,
                accum_out=rr_part[:])
            nc.tensor.matmul(gam_ps[:], bc128[:], rr_part[:])
            nc.vector.tensor_mul(bvec[:], gam_ps[:], recg[:])
            nc.scalar.copy(gamvec[:], gam_ps[:])

            # p = (p * beta) + r in one pass, then refresh halos
            nc.vector.scalar_tensor_tensor(
                p_c2, p_c2, bvec[:], r[:], Alu.mult, Alu.add)
            halo_update()

        if loop_mode == "plain":
            with tc.For_i(0, iters) as _i:
                body(_i)
        elif loop_mode == "stag":
            with tc.For_i(0, iters, staggered_reset=True) as _i:
                body(_i)
        elif loop_mode.startswith("unroll"):
            tc.For_i_unrolled(0, iters, 1, body, max_unroll=int(loop_mode[6:]))
        else:
            raise ValueError(loop_mode)

        nc.sync.dma_start(x_d, x[:])

    nc.compile()
    return nc


VARIANT = os.environ.get("KERNEL_VARIANT", "v3")


def _get_nc(iters, variant=None):
    variant = variant or VARIANT
    key = ("nc", iters, variant, os.environ.get("KERNEL_LOOP", "unroll4"),
           os.environ.get("KERNEL_SPLITS", ""), os.environ.get("KERNEL_KPROD", ""),
           os.environ.get("KERNEL_XUPD", ""), os.environ.get("KERNEL_RR", ""))
    if key not in _CACHE:
        builder = {"std": _build_nc, "qrec": _build_nc_qrec, "v2": _build_nc_v2,
                   "v3": _build_nc_v3}[variant]
        _CACHE[key] = builder(iters)
    return _CACHE[key]


def _expected_inputs(nc):
    import concourse.mybir as mybir
    part = nc.partition_id_tensor.name if nc.partition_id_tensor else None
    names = set()
    for alloc in nc.m.functions[0].allocations:
        if isinstance(alloc, mybir.MemoryLocationSet) and alloc.kind == "ExternalInput":
            nm = alloc.memorylocations[0].name
            if nm != part:
                names.add(nm)
    return names


# ------------------------------------------------------------------- runner

def _make_runner(iters, variant=None):
    """Build the 8-core sharded jit once; returns run(in_maps) -> [x_out]*8."""
    import jax
    from jax.sharding import Mesh, PartitionSpec
    from jax.experimental.shard_map import shard_map
    from concourse import bass2jax, mybir

    nc = _get_nc(iters, variant)
    bass2jax.install_neuronx_cc_hook()
    partition_name = nc.partition_id_tensor.name if nc.partition_id_tensor else None
    in_names, out_names, out_avals, zero_outs = [], [], [], []
    for alloc in nc.m.functions[0].allocations:
        if not isinstance(alloc, mybir.MemoryLocationSet):
            continue
        name = alloc.memorylocations[0].name
        if alloc.kind == "ExternalInput":
            if name != partition_name:
                in_names.append(name)
        elif alloc.kind == "ExternalOutput":
            out_names.append(name)
            shape = tuple(alloc.tensor_shape)
            dtype = mybir.dt.np(alloc.dtype)
            out_avals.append(jax.core.ShapedArray(shape, dtype))
            zero_outs.append(np.zeros(shape, dtype))
    n_params = len(in_names)
    all_in = in_names + out_names + ([partition_name] if partition_name else [])

    def _body(*args):
        ops = list(args)
        if partition_name:
            ops.append(bass2jax.partition_id_tensor())
        return tuple(bass2jax._bass_exec_p.bind(
            *ops, out_avals=tuple(out_avals), in_names=tuple(all_in),
            out_names=tuple(out_names), lowering_input_output_aliases=(),
            sim_require_finite=True, sim_require_nnan=True, nc=nc))

    mesh = Mesh(np.asarray(jax.devices()[:NCORES]), ("core",))
    jf = jax.jit(
        shard_map(_body, mesh=mesh,
                  in_specs=(PartitionSpec("core"),) * (n_params + len(out_names)),
                  out_specs=(PartitionSpec("core"),) * len(out_names),
                  check_rep=False),
        donate_argnums=tuple(range(n_params, n_params + len(out_names))),
        keep_unused=True)

    def prepare(in_maps):
        import jax
        concat_in = [np.concatenate([m[nm] for m in in_maps], axis=0)
                     for nm in in_names]
        dev_in = [jax.device_put(a) for a in concat_in]
        jax.block_until_ready(dev_in)
        return dev_in

    def run_dev(dev_in, fetch=True):
        import jax
        zeros = [np.zeros((NCORES * z.shape[0], *z.shape[1:]), z.dtype)
                 for z in zero_outs]
        outs = jf(*dev_in, *zeros)
        if not fetch:
            jax.block_until_ready(outs)
            return None
        xo = np.asarray(outs[0])
        per_core_rows = xo.shape[0] // NCORES
        return [xo[c * per_core_rows:(c + 1) * per_core_rows] for c in range(NCORES)]

    def run(in_maps):
        return run_dev(prepare(in_maps))

    run.prepare = prepare
    run.run_dev = run_dev
    return run


def _get_runner(iters, variant=None):
    variant = variant or VARIANT
    key = ("runner", iters, variant, os.environ.get("KERNEL_LOOP", "unroll4"),
           os.environ.get("KERNEL_SPLITS", ""), os.environ.get("KERNEL_KPROD", ""),
           os.environ.get("KERNEL_XUPD", ""), os.environ.get("KERNEL_RR", ""))
    if key not in _CACHE:
        _CACHE[key] = _make_runner(iters, variant)
    return _CACHE[key]


def _run(in_maps, iters, variant=None):
    return _get_runner(iters, variant)(in_maps)


def kernel(alpha, f_rhs):
    alpha = np.asarray(alpha, np.float32)
    f_rhs = np.asarray(f_rhs, np.float32)
    in_maps = [_pack_core(alpha[c * BPC:(c + 1) * BPC], f_rhs)
               for c in range(NCORES)]
    try:
        outs = _run(in_maps, ITERS)
    except Exception:
        # a crashed prior session can leave a core wedged; one retry clears it
        outs = _run(in_maps, ITERS)
    unpack = _from_dev_pad if VARIANT == "v3" else _from_dev
    return np.concatenate([unpack(o) for o in outs], axis=0)



# revision 21
# speedup vs baseline: 1.1910x; 1.1910x over previous
"""Trainium2 Bass kernel for batched 2D variable-coefficient diffusion CG solve.

Problem: 64 independent solves of A(alpha) u = f_rhs on a 256x256 grid,
5-point stencil with edge coefficients exp(0.5*(alpha_a + alpha_b)), solved
with 300 fp32 CG iterations (the reference's jax CG never converges before
maxiter=300 -- the 300th iterate is still 1.34x away from the true solution
and moves ~15% per 50 iterations, so the output is an exact trajectory
snapshot: N=290 already exceeds the 2e-2 gate, f64 at N=300 gives 5e-2, and
any fp32 reimplementation lands ~1.5-1.8e-2 away purely from rounding).

Sharding: pure data parallel, 8 problems per NeuronCore across 8 cores.

Default variant "v3": flux-form stencil in a padded-column layout.
Partition P = b*16 + kb; each partition holds 16 k-columns of S=258 slots
(256 j-rows + 2 zero pads). The pads supply the Dirichlet/Neumann zeros so
all 7 stencil passes are flat contiguous window ops on DVE:
    dj = p - p(<<1); Fj = cJF*dj; qj = Fj - Fj(>>1)          (j-direction)
    dk = p - p(<<S); Fk = cKF*dk; q = (qj + Fk_lo) - Fk_hi   (k-direction)
cJF face 0 holds 2*s*KL[0] (Dirichlet), interior faces s*KL[j] (== s*KR[j-1]
exactly). Dot products (pAp, gamma0) run flat over the padded range -- pad
terms are exact +0.0 so the accumulation stream matches the unpadded order
bit-for-bit. r/p updates use pad-skipping 3D views so the pads stay zero.

Engine facts that shaped this (measured via microbench2.py, slope method):
  - fp32 DVE TT/STT pass on [128,~4.2K] is ~4.4-4.7us; accum_out is free;
    the kernel is DVE-throughput-bound at 11 passes/iter (= measured 49us).
  - GpSimd elementwise is fully SERIAL with DVE (shared SBUF port): the old
    kernel's 2 GpSimd products + x-add were additive wall time. Removed.
  - ACT overlaps DVE for free (own port): ||r||^2 runs as ACT Square+accum
    (bit-identical accumulation to the DVE STT accum -- verified on HW) while
    DVE does the x axpy in the same window.
  - Halo copies write only the 256 real slots so the next j-stencil pass
    doesn't falsely depend on the halo round trip (it reads only the
    always-zero pad slot of halo_lo).
  - PE does the per-problem dot segment-sum/broadcast (bc128 matmul) and the
    two 1-column halo shift matmuls. Loop is For_i unrolled x8.

Per iteration, DVE: dj,Fj,qj,dk,Fk,qadd,qsub + pAp(STT+accum) + r axpy(STT,
3D) + x axpy(STT) + p axpy(STT, 3D); ACT: rr Square+accum + 2 halo copies;
PE: 2 reduce + 2 halo matmuls. alpha chain: aneg=(-gamma)*recp (one mul
after the PE trip; bit-equal to -(gamma*recp)).

Measured on trn2 (8 cores), slope method (300 vs 2400 iters):
  baseline (GpSimd split, op-form stencil): 77.9 us/iter = 23.4 ms, err 1.669e-2
  v2 (no GpSimd, contiguous coeffs, x-STT): 64.6 us/iter = 19.4 ms, err 1.669e-2
    (bit-identical trajectory to baseline -- engine moves of elementwise fp32
     ops and reorderings are bit-exact; verified in CoreSim + on HW)
  v3 (flux stencil, 9->7 passes):           55.4 us/iter = 16.6 ms, err 1.522e-2
  + rr->ACT overlap + body reorder:         53.4 us/iter = 16.0 ms, err 1.522e-2
  + halo false-dep fix + unroll8:           49.3 us/iter = 14.8 ms, err 1.522e-2
Measured-and-reverted: unroll16 (54.5), GpSimd tail-splits of DVE passes
(split4 microbench shows DVE+GpSimd serialize), rr->ACT without reorder
(no win at unroll4), custom fused DVE ops (crash under this runtime).
"""

import os
import numpy as np

M = 256
B = 64
NCORES = 8
BPC = B // NCORES          # problems per core
HINV2 = np.float32(M * M)  # exact power of two: folding into coeffs is exact
ITERS = 300
COLS = 16                  # k-columns per partition
F = COLS * M               # 4096 free elements per field
FH = F + 2 * M             # p buffer with halo columns

_CACHE = {}


# ----------------------------------------------------------------- host side

def _coeff_arrays(alpha):
    """Per-problem stencil coefficient fields, matching reference._stencil_coeffs
    fp32 op-for-op, with HINV2 folded in (exact) and off-diagonals negated.

    alpha: (B, 257, 257) f32. Returns diag, KL, KB as (B, 256, 256) f32 where
    KL/KB are the *unmasked-left* / *masked-bottom* edge coefficients."""
    a = alpha.astype(np.float32)
    m = M
    j = np.arange(m)[:, None]
    k = np.arange(m)[None, :]
    KL = np.exp(np.float32(0.5) * (a[:, :-1, :-1] + a[:, :-1, 1:])).astype(np.float32)
    KR = np.where(j < m - 1,
                  np.exp(np.float32(0.5) * (a[:, 1:, :-1] + a[:, 1:, 1:])),
                  np.float32(0.0)).astype(np.float32)
    KB = np.where(k > 0,
                  np.exp(np.float32(0.5) * (a[:, :-1, :-1] + a[:, 1:, :-1])),
                  np.float32(0.0)).astype(np.float32)
    KT = np.where(k < m - 1,
                  np.exp(np.float32(0.5) * (a[:, :-1, 1:] + a[:, 1:, 1:])),
                  np.float32(0.0)).astype(np.float32)
    diag = KL + KR + KB + KT + np.where(j == 0, KL, np.float32(0.0)).astype(np.float32)
    return diag, KL, KB


def _to_dev(arr_bjk):
    """(BPC, 256j, 256k) -> [128, 4096] with P = b*16+kb, free = c*256+j."""
    t = arr_bjk.transpose(0, 2, 1)                 # (b, k, j)
    t = t.reshape(BPC, 16, COLS, M)                # (b, kb, c, j)
    return np.ascontiguousarray(t.reshape(128, F))


def _from_dev(dev):
    """[128, 4096] -> (BPC, 256j, 256k)."""
    t = dev.reshape(BPC, 16, COLS, M).transpose(0, 3, 1, 2)   # (b, j, kb, c)
    return np.ascontiguousarray(t.reshape(BPC, M, M))


def _from_dev_pad(dev):
    """[128, COLS*(M+2)] padded -> (BPC, 256j, 256k)."""
    t = dev.reshape(BPC, 16, COLS, M + 2)[:, :, :, :M].transpose(0, 3, 1, 2)
    return np.ascontiguousarray(t.reshape(BPC, M, M))


def _pack_core(alpha_core, f_rhs):
    """Build the per-core input map (all fp32 numpy arrays)."""
    diag, KL, KB = _coeff_arrays(alpha_core)
    s = HINV2
    cD = _to_dev(diag * s)                               # [128, 4096]
    nKL = _to_dev(KL * (-s)).reshape(128, COLS, M)       # (P, c, j)
    nKB = _to_dev(KB * (-s)).reshape(128, COLS, M)

    # cLp[P, c, 0..256]: 0 at jj=0 (Dirichlet kill for the j-1 shift),
    # -s*KL[jj,k] at jj=1..255, 0 at jj=256 (K_right mask at j=255).
    cLp = np.zeros((128, COLS, M + 1), np.float32)
    cLp[:, :, 1:M] = nKL[:, :, 1:M]

    # v2: contiguous copies of the two strided j-coefficient views
    cJM = np.ascontiguousarray(cLp[:, :, 0:M].reshape(128, F))      # multiplies p_jm1
    cJP = np.ascontiguousarray(cLp[:, :, 1:M + 1].reshape(128, F))  # multiplies p_jp1

    # cBp[P, 0..16, j]: c=0..15 the (already k-masked) bottom coefficients,
    # c=16 the next partition's c=0 column (static k-halo; 0 past k=255).
    cBp = np.zeros((128, COLS + 1, M), np.float32)
    cBp[:, :COLS, :] = nKB
    nKB4 = nKB.reshape(BPC, 16, COLS, M)
    cBp4 = cBp.reshape(BPC, 16, COLS + 1, M)
    cBp4[:, :-1, COLS, :] = nKB4[:, 1:, 0, :]

    fdev = _to_dev(np.broadcast_to(f_rhs, (BPC, M, M)).astype(np.float32))

    # ---- v3 flux-form packing: padded columns (S=258 slots, 2 zero pads)
    S = M + 2
    FP = COLS * S
    def _pad_cols(flat128):                       # [128, F] -> [128, FP]
        t = flat128.reshape(128, COLS, M)
        out = np.zeros((128, COLS, S), np.float32)
        out[:, :, :M] = t
        return out.reshape(128, FP)
    fpad = _pad_cols(fdev)
    xKL = _to_dev((KL * s).astype(np.float32))    # +s*KL unmasked, dev layout
    cJF = _pad_cols(xKL)
    cJF3 = cJF.reshape(128, COLS, S)
    cJF3[:, :, 0] = (np.float32(2.0) * xKL.reshape(128, COLS, M)[:, :, 0])
    # j-faces 256..257 are pad faces = 0; face 256 would be s*KR[255] = 0 anyway
    cJF = np.ascontiguousarray(cJF3.reshape(128, FP))
    # k-faces: 17 per partition, +s*K = -cBp (cBp holds -s*K, k-masked, with halo col)
    cKF = np.zeros((128, COLS + 1, S), np.float32)
    cKF[:, :, :M] = -cBp
    cKF = np.ascontiguousarray(cKF.reshape(128, (COLS + 1) * S))

    seg = np.zeros((128, BPC), np.float32)               # seg[q, b] = q//16 == b
    seg[np.arange(128), np.arange(128) // 16] = 1.0
    bc = np.ascontiguousarray(seg.T)                     # (8, 128)
    qi = np.arange(128)
    bc128 = (qi[:, None] // 16 == qi[None, :] // 16).astype(np.float32)
    sdn = np.eye(128, 128, 1, np.float32)                # out[i] = in[i-1]
    sup = np.eye(128, 128, -1, np.float32)               # out[i] = in[i+1]

    return {
        "fp_in": fpad,
        "cJF_in": cJF,
        "cKF_in": cKF,
        "f_in": fdev,
        "cD_in": cD,
        "cL_in": np.ascontiguousarray(cLp.reshape(128, COLS * (M + 1))),
        "cJM_in": cJM,
        "cJP_in": cJP,
        "cB_in": np.ascontiguousarray(cBp.reshape(128, (COLS + 1) * M)),
        "seg_in": seg,
        "bc_in": bc,
        "bc128_in": bc128,
        "sdn_in": sdn,
        "sup_in": sup,
    }


# --------------------------------------------------------------- bass kernel

def _build_nc_qrec(iters):
    """q-recurrence variant: q_{k+1} = A r_{k+1} + beta_k q_k.

    The stencil runs on r (available right after the r update), so the
    ||r||^2 / beta / p-update chain hides behind it. Validated in exp3.py:
    lands as close to the f64 trajectory as plain fp32 CG.

    Loop state: p, q (= A p), r (halo'd), x, gamvec ([128,1] per-problem
    gamma broadcast). Body:
        pAp = <p, q>; alpha = gamma/pAp
        x += alpha p ; r -= alpha q ; refresh r halos
        gamma' = ||r||^2 ; beta = gamma'/gamma
        w = A r  (overlaps beta chain and p update)
        p = r + beta p ; q = w + beta q
    """
    from contextlib import ExitStack
    import concourse.bass as bass
    import concourse.tile as tile
    from concourse import bacc, mybir

    f32 = mybir.dt.float32
    Alu = mybir.AluOpType
    Act = mybir.ActivationFunctionType

    nc = bacc.Bacc("TRN2", target_bir_lowering=False, debug=False)

    f_d = nc.dram_tensor("f_in", [128, F], f32, kind="ExternalInput").ap()
    cD_d = nc.dram_tensor("cD_in", [128, F], f32, kind="ExternalInput").ap()
    cL_d = nc.dram_tensor("cL_in", [128, COLS * (M + 1)], f32, kind="ExternalInput").ap()
    cB_d = nc.dram_tensor("cB_in", [128, (COLS + 1) * M], f32, kind="ExternalInput").ap()
    bc128_d = nc.dram_tensor("bc128_in", [128, 128], f32, kind="ExternalInput").ap()
    sdn_d = nc.dram_tensor("sdn_in", [128, 128], f32, kind="ExternalInput").ap()
    sup_d = nc.dram_tensor("sup_in", [128, 128], f32, kind="ExternalInput").ap()
    x_d = nc.dram_tensor("x_out", [128, F], f32, kind="ExternalOutput").ap()

    with tile.TileContext(nc) as tc, ExitStack() as ctx:
        sb = ctx.enter_context(tc.tile_pool(name="state", bufs=1))
        ps = ctx.enter_context(tc.tile_pool(name="psum", bufs=1, space="PSUM"))

        r = sb.tile([128, FH], f32)       # halo_lo | center | halo_hi
        p = sb.tile([128, F], f32)
        x = sb.tile([128, F], f32)
        q = sb.tile([128, F], f32)        # A @ p via recurrence
        t0 = sb.tile([128, F], f32)
        t1 = sb.tile([128, F], f32)
        t2 = sb.tile([128, F], f32)
        t3 = sb.tile([128, F], f32)
        cD = sb.tile([128, F], f32)
        cL = sb.tile([128, COLS * (M + 1)], f32)
        cB = sb.tile([128, (COLS + 1) * M], f32)
        bc128 = sb.tile([128, 128], f32)
        sdn = sb.tile([128, 128], f32)
        sup = sb.tile([128, 128], f32)

        pap_part = sb.tile([128, 1], f32)
        rr_part = sb.tile([128, 1], f32)
        gamvec = sb.tile([128, 1], f32)   # per-problem gamma, broadcast
        recg = sb.tile([128, 1], f32)     # 1/gamma_old
        recp = sb.tile([128, 1], f32)     # 1/pAp
        avec = sb.tile([128, 1], f32)
        bvec = sb.tile([128, 1], f32)

        pap_ps = ps.tile([128, 1], f32)
        gam_ps = ps.tile([128, 1], f32)
        hlo_ps = ps.tile([128, M], f32)
        hhi_ps = ps.tile([128, M], f32)

        def v3(ap2d):
            return ap2d.rearrange("p (c j) -> p c j", c=COLS, j=M)

        r_c2 = r[:, M:M + F]
        r_c3 = v3(r_c2)
        r_jm1 = v3(r[:, M - 1:M - 1 + F])
        r_jp1 = v3(r[:, M + 1:M + 1 + F])
        r_km1 = v3(r[:, 0:F])
        r_kp1 = v3(r[:, 2 * M:2 * M + F])
        cL3 = cL[:].rearrange("p (c j) -> p c j", c=COLS, j=M + 1)
        cLl = cL3[:, :, 0:M]
        cLr = cL3[:, :, 1:M + 1]
        cB3 = cB[:].rearrange("p (c j) -> p c j", c=COLS + 1, j=M)
        cBb = cB3[:, 0:COLS, :]
        cBt = cB3[:, 1:COLS + 1, :]
        cD3 = v3(cD[:])

        nc.sync.dma_start(cD[:], cD_d)
        nc.sync.dma_start(cL[:], cL_d)
        nc.sync.dma_start(cB[:], cB_d)
        nc.sync.dma_start(bc128[:], bc128_d)
        nc.sync.dma_start(sdn[:], sdn_d)
        nc.sync.dma_start(sup[:], sup_d)
        nc.sync.dma_start(r_c2, f_d)
        nc.sync.dma_start(p[:], f_d)

        def halo_update():
            nc.tensor.matmul(hlo_ps[:], sdn[:], r[:, F:F + M])
            nc.tensor.matmul(hhi_ps[:], sup[:], r[:, M:2 * M])
            nc.scalar.copy(r[:, 0:M], hlo_ps[:])
            nc.scalar.copy(r[:, F + M:F + 2 * M], hhi_ps[:])

        def stencil_w():
            """t0 = A @ r (j-terms on DVE, k-products on GpSimd)."""
            nc.gpsimd.tensor_mul(v3(t2[:]), cBb, r_km1)
            nc.gpsimd.tensor_mul(v3(t3[:]), cBt, r_kp1)
            nc.vector.tensor_mul(v3(t0[:]), cD3, r_c3)
            nc.vector.tensor_mul(v3(t1[:]), cLl, r_jm1)
            nc.vector.tensor_add(t0[:], t0[:], t1[:])
            nc.vector.tensor_mul(v3(t1[:]), cLr, r_jp1)
            nc.vector.tensor_add(t0[:], t0[:], t1[:])
            nc.vector.tensor_add(t0[:], t0[:], t2[:])
            nc.vector.tensor_add(t0[:], t0[:], t3[:])

        # ---- init: x=0, r=p=f, q = A p, gamma0
        nc.vector.memset(x[:], 0.0)
        halo_update()
        nc.scalar.activation(t1[:], r_c2, Act.Square, accum_out=rr_part[:])
        nc.tensor.matmul(gam_ps[:], bc128[:], rr_part[:])
        nc.scalar.copy(gamvec[:], gam_ps[:])
        stencil_w()
        nc.vector.tensor_copy(q[:], t0[:])

        # ---- 300 CG iterations
        with tc.For_i(0, iters) as _i:
            nc.vector.reciprocal(recg[:], gamvec[:])

            # pAp and alpha
            nc.vector.tensor_mul(t3[:], p[:], q[:])
            nc.scalar.activation(t3[:], t3[:], Act.Copy, accum_out=pap_part[:])
            nc.tensor.matmul(pap_ps[:], bc128[:], pap_part[:])
            nc.vector.reciprocal(recp[:], pap_ps[:])
            nc.vector.tensor_mul(avec[:], gamvec[:], recp[:])

            # x += alpha*p (ACT+GpSimd, off critical) ; r -= alpha*q (DVE)
            nc.scalar.activation(t2[:], p[:], Act.Copy, scale=avec[:])
            nc.gpsimd.tensor_add(x[:], x[:], t2[:])
            nc.vector.tensor_scalar_mul(t1[:], q[:], avec[:])
            nc.vector.tensor_sub(r_c2, r_c2, t1[:])
            halo_update()

            # gamma' and beta (hidden under the stencil)
            nc.scalar.activation(t1[:], r_c2, Act.Square, accum_out=rr_part[:])
            nc.tensor.matmul(gam_ps[:], bc128[:], rr_part[:])
            nc.vector.tensor_mul(bvec[:], gam_ps[:], recg[:])
            nc.scalar.copy(gamvec[:], gam_ps[:])

            # w = A r
            stencil_w()

            # p = r + beta*p (GpSimd) ; q = w + beta*q (DVE)
            nc.gpsimd.tensor_scalar_mul(t2[:], p[:], bvec[:])
            nc.gpsimd.tensor_add(p[:], r_c2, t2[:])
            nc.vector.tensor_scalar_mul(t1[:], q[:], bvec[:])
            nc.vector.tensor_add(q[:], t0[:], t1[:])

        nc.sync.dma_start(x_d, x[:])

    nc.compile()
    return nc


def _build_nc_v2(iters):
    """Bit-exact restructuring of the std kernel:

    - strided cLl/cLr coefficient views replaced by contiguous cJM/cJP arrays
    - every field op is a flat 2D contiguous window (no 3D APs on DVE)
    - alpha chain tightened: aneg = (-gamma)*recp directly (one op after PE)
    - optional GpSimd tail-splits on the j-stencil/add ops (SPLITS dict),
      bit-exact since IEEE elementwise ops match across engines
    """
    from contextlib import ExitStack
    import concourse.bass as bass
    import concourse.tile as tile
    from concourse import bacc, mybir
    import json

    f32 = mybir.dt.float32
    Alu = mybir.AluOpType
    Act = mybir.ActivationFunctionType

    # gp tail elements per op site (0 = whole op on DVE)
    SP = {"mcd": 0, "mjm": 0, "a1": 0, "mjp": 0, "a2": 0, "a3": 0, "a4": 0,
          "rup": 0, "pup": 0}
    env = os.environ.get("KERNEL_SPLITS")
    if env:
        SP.update(json.loads(env))
    # engine modes: kprod = dve|gp (the 2 k-shift products), xupd = stt|actgp
    KPROD = os.environ.get("KERNEL_KPROD", "dve")
    XUPD = os.environ.get("KERNEL_XUPD", "stt")
    RRENG = os.environ.get("KERNEL_RR", "dve")  # dve|act (act changes accum order!)

    nc = bacc.Bacc("TRN2", target_bir_lowering=False, debug=False)

    f_d = nc.dram_tensor("f_in", [128, F], f32, kind="ExternalInput").ap()
    cD_d = nc.dram_tensor("cD_in", [128, F], f32, kind="ExternalInput").ap()
    cJM_d = nc.dram_tensor("cJM_in", [128, F], f32, kind="ExternalInput").ap()
    cJP_d = nc.dram_tensor("cJP_in", [128, F], f32, kind="ExternalInput").ap()
    cB_d = nc.dram_tensor("cB_in", [128, (COLS + 1) * M], f32, kind="ExternalInput").ap()
    bc128_d = nc.dram_tensor("bc128_in", [128, 128], f32, kind="ExternalInput").ap()
    sdn_d = nc.dram_tensor("sdn_in", [128, 128], f32, kind="ExternalInput").ap()
    sup_d = nc.dram_tensor("sup_in", [128, 128], f32, kind="ExternalInput").ap()
    x_d = nc.dram_tensor("x_out", [128, F], f32, kind="ExternalOutput").ap()

    with tile.TileContext(nc) as tc, ExitStack() as ctx:
        sb = ctx.enter_context(tc.tile_pool(name="state", bufs=1))
        ps = ctx.enter_context(tc.tile_pool(name="psum", bufs=1, space="PSUM"))

        p = sb.tile([128, FH], f32)       # halo_lo | center | halo_hi
        r = sb.tile([128, F], f32)
        x = sb.tile([128, F], f32)
        q = sb.tile([128, F], f32)
        t0 = sb.tile([128, F], f32)
        t1 = sb.tile([128, F], f32)
        t2 = sb.tile([128, F], f32)
        t3 = sb.tile([128, F], f32)
        cD = sb.tile([128, F], f32)
        cJM = sb.tile([128, F], f32)
        cJP = sb.tile([128, F], f32)
        cB = sb.tile([128, (COLS + 1) * M], f32)
        bc128 = sb.tile([128, 128], f32)
        sdn = sb.tile([128, 128], f32)
        sup = sb.tile([128, 128], f32)

        pap_part = sb.tile([128, 1], f32)
        rr_part = sb.tile([128, 1], f32)
        gamvec = sb.tile([128, 1], f32)
        gneg = sb.tile([128, 1], f32)     # -gamma (for aneg, off critical)
        recg = sb.tile([128, 1], f32)
        recp = sb.tile([128, 1], f32)
        avec = sb.tile([128, 1], f32)
        aneg = sb.tile([128, 1], f32)
        bvec = sb.tile([128, 1], f32)

        pap_ps = ps.tile([128, 1], f32)
        gam_ps = ps.tile([128, 1], f32)
        hlo_ps = ps.tile([128, M], f32)
        hhi_ps = ps.tile([128, M], f32)

        # flat contiguous windows
        pc = p[:, M:M + F]
        pjm = p[:, M - 1:M - 1 + F]
        pjp = p[:, M + 1:M + 1 + F]
        pkm = p[:, 0:F]
        pkp = p[:, 2 * M:2 * M + F]
        cBb = cB[:, 0:F]
        cBt = cB[:, M:M + F]

        # ---- load inputs
        nc.sync.dma_start(cD[:], cD_d)
        nc.sync.dma_start(cJM[:], cJM_d)
        nc.sync.dma_start(cJP[:], cJP_d)
        nc.sync.dma_start(cB[:], cB_d)
        nc.sync.dma_start(bc128[:], bc128_d)
        nc.sync.dma_start(sdn[:], sdn_d)
        nc.sync.dma_start(sup[:], sup_d)
        nc.sync.dma_start(r[:], f_d)
        nc.sync.dma_start(pc, f_d)

        def halo_update():
            nc.tensor.matmul(hlo_ps[:], sdn[:], p[:, F:F + M])
            nc.tensor.matmul(hhi_ps[:], sup[:], p[:, M:2 * M])
            nc.scalar.copy(p[:, 0:M], hlo_ps[:])
            nc.scalar.copy(p[:, F + M:F + 2 * M], hhi_ps[:])

        def mul2(site, out, in0, in1):
            """TT mul with optional GpSimd tail split (bit-exact)."""
            s = SP[site]
            if s:
                nc.vector.tensor_mul(out[:, :F - s], in0[:, :F - s], in1[:, :F - s])
                nc.gpsimd.tensor_mul(out[:, F - s:], in0[:, F - s:], in1[:, F - s:])
            else:
                nc.vector.tensor_mul(out, in0, in1)

        def add2(site, out, in0, in1):
            s = SP[site]
            if s:
                nc.vector.tensor_add(out[:, :F - s], in0[:, :F - s], in1[:, :F - s])
                nc.gpsimd.tensor_add(out[:, F - s:], in0[:, F - s:], in1[:, F - s:])
            else:
                nc.vector.tensor_add(out, in0, in1)

        # ---- init: x=0, gamma0 = per-problem ||f||^2, p halos
        nc.vector.memset(x[:], 0.0)
        halo_update()
        nc.scalar.activation(t1[:], r[:], Act.Square, accum_out=rr_part[:])
        nc.tensor.matmul(gam_ps[:], bc128[:], rr_part[:])
        nc.scalar.copy(gamvec[:], gam_ps[:])
        nc.vector.tensor_scalar_mul(gneg[:], gamvec[:], -1.0)

        # ---- 300 CG iterations
        def body(_i):
            nc.vector.reciprocal(recg[:], gamvec[:])

            # q = A @ p
            if KPROD == "gp":
                nc.gpsimd.tensor_mul(t2[:], cBb, pkm)
                nc.gpsimd.tensor_mul(t3[:], cBt, pkp)
            else:
                nc.vector.tensor_mul(t2[:], cBb, pkm)
                nc.vector.tensor_mul(t3[:], cBt, pkp)
            mul2("mcd", t0, cD[:], pc)
            mul2("mjm", t1, cJM[:], pjm)
            add2("a1", t0, t0[:], t1[:])
            mul2("mjp", t1, cJP[:], pjp)
            add2("a2", t0, t0[:], t1[:])
            add2("a3", t0, t0[:], t2[:])
            add2("a4", q, t0[:], t3[:])

            # pAp = sum(p*q) fused in one DVE pass; alpha = gamma/pAp
            nc.vector.scalar_tensor_tensor(
                t3[:], pc, 1.0, q[:], Alu.mult, Alu.mult,
                accum_out=pap_part[:])
            nc.tensor.matmul(pap_ps[:], bc128[:], pap_part[:])
            nc.vector.reciprocal(recp[:], pap_ps[:])
            # aneg = (-gamma) * recp  == -(gamma*recp) bit-exactly
            nc.vector.tensor_mul(aneg[:], gneg[:], recp[:])

            # r = (q * -alpha) + r, one pass
            s = SP["rup"]
            if s:
                nc.vector.scalar_tensor_tensor(
                    r[:, :F - s], q[:, :F - s], aneg[:], r[:, :F - s],
                    Alu.mult, Alu.add)
                nc.gpsimd.tensor_scalar_mul(t0[:, F - s:], q[:, F - s:], aneg[:])
                nc.gpsimd.tensor_add(r[:, F - s:], t0[:, F - s:], r[:, F - s:])
            else:
                nc.vector.scalar_tensor_tensor(
                    r[:], q[:], aneg[:], r[:], Alu.mult, Alu.add)

            # x += alpha*p off-critical
            nc.vector.tensor_mul(avec[:], gamvec[:], recp[:])
            if XUPD == "actgp":
                nc.scalar.activation(t1[:], pc, Act.Copy, scale=avec[:])
                nc.gpsimd.tensor_add(x[:], x[:], t1[:])
            else:  # single DVE STT: x = (p*alpha) + x, bit-exact same values
                nc.vector.scalar_tensor_tensor(
                    x[:], pc, avec[:], x[:], Alu.mult, Alu.add)

            # gamma' = sum(r*r); beta
            if RRENG == "act":
                nc.scalar.activation(t2[:], r[:], Act.Square, accum_out=rr_part[:])
            else:
                nc.vector.scalar_tensor_tensor(
                    t2[:], r[:], 1.0, r[:], Alu.mult, Alu.mult,
                    accum_out=rr_part[:])
            nc.tensor.matmul(gam_ps[:], bc128[:], rr_part[:])
            nc.vector.tensor_mul(bvec[:], gam_ps[:], recg[:])
            nc.scalar.copy(gamvec[:], gam_ps[:])
            nc.vector.tensor_scalar_mul(gneg[:], gamvec[:], -1.0)

            # p = (p * beta) + r in one pass, then refresh halos
            s = SP["pup"]
            if s:
                nc.vector.scalar_tensor_tensor(
                    pc[:, :F - s], pc[:, :F - s], bvec[:], r[:, :F - s],
                    Alu.mult, Alu.add)
                nc.gpsimd.tensor_scalar_mul(t2[:, F - s:], pc[:, F - s:], bvec[:])
                nc.gpsimd.tensor_add(pc[:, F - s:], t2[:, F - s:], r[:, F - s:])
            else:
                nc.vector.scalar_tensor_tensor(
                    pc, pc, bvec[:], r[:], Alu.mult, Alu.add)
            halo_update()

        loop_mode = os.environ.get("KERNEL_LOOP", "unroll4")
        if loop_mode == "plain":
            with tc.For_i(0, iters) as _i:
                body(_i)
        elif loop_mode == "stag":
            with tc.For_i(0, iters, staggered_reset=True) as _i:
                body(_i)
        elif loop_mode.startswith("unroll"):
            tc.For_i_unrolled(0, iters, 1, body, max_unroll=int(loop_mode[6:]))
        else:
            raise ValueError(loop_mode)

        nc.sync.dma_start(x_d, x[:])

    nc.compile()
    return nc


def _build_nc_v3(iters):
    """Flux-form stencil in a padded-column layout: 7 stencil passes vs 9.

    Each column holds S=258 slots (256 rows + 2 zero pads). The pads supply
    the Dirichlet/Neumann zeros so every stencil op is a flat contiguous
    window subtract/multiply:
        dj = p - p(<<1); Fj = cJF*dj; qj = Fj - Fj(>>1)
        dk = p - p(<<S); Fk = cKF*dk; q = (qj + Fk_lo) - Fk_hi
    Dot products run flat over the padded range (pad terms are exact +0.0,
    so the accumulation stream matches the unpadded order bit-for-bit);
    r/p updates use pad-skipping 3D views so the pads stay exactly zero.
    Trajectory note: the flux association differs from the operator form;
    measured on CPU at 1.66e-2 vs reference (same class as op form).
    """
    from contextlib import ExitStack
    import concourse.bass as bass
    import concourse.tile as tile
    from concourse import bacc, mybir

    f32 = mybir.dt.float32
    Alu = mybir.AluOpType
    Act = mybir.ActivationFunctionType

    S = M + 2
    FP = COLS * S
    FHP = FP + 2 * S
    KFP = (COLS + 1) * S
    RRENG = os.environ.get("KERNEL_RR", "act")

    nc = bacc.Bacc("TRN2", target_bir_lowering=False, debug=False)

    fp_d = nc.dram_tensor("fp_in", [128, FP], f32, kind="ExternalInput").ap()
    cJF_d = nc.dram_tensor("cJF_in", [128, FP], f32, kind="ExternalInput").ap()
    cKF_d = nc.dram_tensor("cKF_in", [128, KFP], f32, kind="ExternalInput").ap()
    bc128_d = nc.dram_tensor("bc128_in", [128, 128], f32, kind="ExternalInput").ap()
    sdn_d = nc.dram_tensor("sdn_in", [128, 128], f32, kind="ExternalInput").ap()
    sup_d = nc.dram_tensor("sup_in", [128, 128], f32, kind="ExternalInput").ap()
    x_d = nc.dram_tensor("xp_out", [128, FP], f32, kind="ExternalOutput").ap()

    with tile.TileContext(nc) as tc, ExitStack() as ctx:
        sb = ctx.enter_context(tc.tile_pool(name="state", bufs=1))
        ps = ctx.enter_context(tc.tile_pool(name="psum", bufs=1, space="PSUM"))

        p = sb.tile([128, FHP], f32)     # halo_lo(S) | center(FP) | halo_hi(S)
        r = sb.tile([128, FP], f32)
        x = sb.tile([128, FP], f32)
        q = sb.tile([128, FP], f32)
        t0 = sb.tile([128, FP], f32)
        t1 = sb.tile([128, FP + 8], f32)
        t2 = sb.tile([128, KFP], f32)
        cJF = sb.tile([128, FP], f32)
        cKF = sb.tile([128, KFP], f32)
        bc128 = sb.tile([128, 128], f32)
        sdn = sb.tile([128, 128], f32)
        sup = sb.tile([128, 128], f32)

        pap_part = sb.tile([128, 1], f32)
        rr_part = sb.tile([128, 1], f32)
        gamvec = sb.tile([128, 1], f32)
        gneg = sb.tile([128, 1], f32)
        recg = sb.tile([128, 1], f32)
        recp = sb.tile([128, 1], f32)
        avec = sb.tile([128, 1], f32)
        aneg = sb.tile([128, 1], f32)
        bvec = sb.tile([128, 1], f32)

        pap_ps = ps.tile([128, 1], f32)
        gam_ps = ps.tile([128, 1], f32)
        hlo_ps = ps.tile([128, S], f32)
        hhi_ps = ps.tile([128, S], f32)

        pcen = p[:, S:S + FP]

        def vv(ap2d):
            """pad-skipping 3D view over a [128, FP] range."""
            return ap2d.rearrange("p (c j) -> p c j", c=COLS, j=S)[:, :, 0:M]

        # ---- load inputs / init
        nc.sync.dma_start(cJF[:], cJF_d)
        nc.sync.dma_start(cKF[:], cKF_d)
        nc.sync.dma_start(bc128[:], bc128_d)
        nc.sync.dma_start(sdn[:], sdn_d)
        nc.sync.dma_start(sup[:], sup_d)
        nc.vector.memset(p[:], 0.0)
        nc.vector.memset(x[:], 0.0)
        nc.vector.memset(t1[:], 0.0)
        nc.sync.dma_start(r[:], fp_d)
        nc.sync.dma_start(pcen, fp_d)

        def halo_update():
            # copy only the 256 real slots: pads stay 0 from init, and the
            # j-stencil's read of the always-zero pad slot S-1 no longer
            # falsely depends on this round trip
            nc.tensor.matmul(hlo_ps[:, 0:M], sdn[:], p[:, FP:FP + M])
            nc.tensor.matmul(hhi_ps[:, 0:M], sup[:], p[:, S:S + M])
            nc.scalar.copy(p[:, 0:M], hlo_ps[:, 0:M])
            nc.scalar.copy(p[:, S + FP:S + FP + M], hhi_ps[:, 0:M])

        halo_update()
        nc.scalar.activation(t0[:], r[:], Act.Square, accum_out=rr_part[:])
        nc.tensor.matmul(gam_ps[:], bc128[:], rr_part[:])
        nc.scalar.copy(gamvec[:], gam_ps[:])
        nc.vector.tensor_scalar_mul(gneg[:], gamvec[:], -1.0)

        def body(_i):
            nc.vector.reciprocal(recg[:], gamvec[:])

            # q = A p, flux form: 7 flat passes
            nc.vector.tensor_sub(t1[:, 0:FP], pcen, p[:, S - 1:S - 1 + FP])
            nc.vector.tensor_mul(t1[:, 0:FP], cJF[:], t1[:, 0:FP])
            nc.vector.tensor_sub(t0[:], t1[:, 0:FP], t1[:, 1:FP + 1])
            nc.vector.tensor_sub(t2[:], p[:, S:S + KFP], p[:, 0:KFP])
            nc.vector.tensor_mul(t2[:], cKF[:], t2[:])
            nc.vector.tensor_add(q[:], t0[:], t2[:, 0:FP])
            nc.vector.tensor_sub(q[:], q[:], t2[:, S:S + FP])

            # pAp (flat; pad terms are exact zeros) ; alpha = gamma/pAp
            nc.vector.scalar_tensor_tensor(
                t0[:], pcen, 1.0, q[:], Alu.mult, Alu.mult,
                accum_out=pap_part[:])
            nc.tensor.matmul(pap_ps[:], bc128[:], pap_part[:])
            nc.vector.reciprocal(recp[:], pap_ps[:])
            nc.vector.tensor_mul(aneg[:], gneg[:], recp[:])

            # r = (q * -alpha) + r  (pad-skipping: r pads stay 0)
            nc.vector.scalar_tensor_tensor(
                vv(r[:]), vv(q[:]), aneg[:], vv(r[:]), Alu.mult, Alu.add)

            # gamma' = ||r||^2 immediately after r (flat; pads exact zeros)
            if RRENG == "act":
                nc.scalar.activation(t2[:, 0:FP], r[:], Act.Square,
                                     accum_out=rr_part[:])
            else:
                nc.vector.scalar_tensor_tensor(
                    t2[:, 0:FP], r[:], 1.0, r[:], Alu.mult, Alu.mult,
                    accum_out=rr_part[:])
            nc.tensor.matmul(gam_ps[:], bc128[:], rr_part[:])

            # x += alpha*p fills the gamma PE-trip window (flat; pads stay 0)
            nc.vector.tensor_mul(avec[:], gamvec[:], recp[:])
            nc.vector.scalar_tensor_tensor(
                x[:], pcen, avec[:], x[:], Alu.mult, Alu.add)

            nc.vector.tensor_mul(bvec[:], gam_ps[:], recg[:])
            nc.scalar.copy(gamvec[:], gam_ps[:])
            nc.vector.tensor_scalar_mul(gneg[:], gamvec[:], -1.0)

            # p = (p * beta) + r (pad-skipping: p pads stay 0), then halos
            nc.vector.scalar_tensor_tensor(
                vv(pcen), vv(pcen), bvec[:], vv(r[:]), Alu.mult, Alu.add)
            halo_update()

        loop_mode = os.environ.get("KERNEL_LOOP", "unroll8")
        if loop_mode == "plain":
            with tc.For_i(0, iters) as _i:
                body(_i)
        elif loop_mode.startswith("unroll"):
            tc.For_i_unrolled(0, iters, 1, body, max_unroll=int(loop_mode[6:]))
        else:
            raise ValueError(loop_mode)

        nc.sync.dma_start(x_d, x[:])

    nc.compile()
    return nc


def _build_nc(iters):
    from contextlib import ExitStack
    import concourse.bass as bass
    import concourse.tile as tile
    from concourse import bacc, mybir

    f32 = mybir.dt.float32
    Alu = mybir.AluOpType
    Act = mybir.ActivationFunctionType

    nc = bacc.Bacc("TRN2", target_bir_lowering=False, debug=False)

    f_d = nc.dram_tensor("f_in", [128, F], f32, kind="ExternalInput").ap()
    cD_d = nc.dram_tensor("cD_in", [128, F], f32, kind="ExternalInput").ap()
    cL_d = nc.dram_tensor("cL_in", [128, COLS * (M + 1)], f32, kind="ExternalInput").ap()
    cB_d = nc.dram_tensor("cB_in", [128, (COLS + 1) * M], f32, kind="ExternalInput").ap()
    bc128_d = nc.dram_tensor("bc128_in", [128, 128], f32, kind="ExternalInput").ap()
    sdn_d = nc.dram_tensor("sdn_in", [128, 128], f32, kind="ExternalInput").ap()
    sup_d = nc.dram_tensor("sup_in", [128, 128], f32, kind="ExternalInput").ap()
    x_d = nc.dram_tensor("x_out", [128, F], f32, kind="ExternalOutput").ap()

    with tile.TileContext(nc) as tc, ExitStack() as ctx:
        sb = ctx.enter_context(tc.tile_pool(name="state", bufs=1))
        ps = ctx.enter_context(tc.tile_pool(name="psum", bufs=1, space="PSUM"))

        p = sb.tile([128, FH], f32)       # halo_lo | center | halo_hi
        r = sb.tile([128, F], f32)
        x = sb.tile([128, F], f32)
        q = sb.tile([128, F], f32)        # A @ p
        t0 = sb.tile([128, F], f32)       # DVE stencil accumulator
        t1 = sb.tile([128, F], f32)       # DVE-only scratch (products, axpy terms)
        t2 = sb.tile([128, F], f32)       # GpSimd m3 product / ACT rr junk
        t3 = sb.tile([128, F], f32)       # GpSimd m4 product / pAp product / x term
        t4 = sb.tile([128, F], f32)       # GpSimd m1 product (dedicated)
        cD = sb.tile([128, F], f32)
        cL = sb.tile([128, COLS * (M + 1)], f32)
        cB = sb.tile([128, (COLS + 1) * M], f32)
        bc128 = sb.tile([128, 128], f32)
        sdn = sb.tile([128, 128], f32)
        sup = sb.tile([128, 128], f32)

        pap_part = sb.tile([128, 1], f32)
        rr_part = sb.tile([128, 1], f32)
        gamvec = sb.tile([128, 1], f32)   # per-problem gamma, broadcast
        recg = sb.tile([128, 1], f32)
        recp = sb.tile([128, 1], f32)
        avec = sb.tile([128, 1], f32)
        aneg = sb.tile([128, 1], f32)
        bvec = sb.tile([128, 1], f32)

        pap_ps = ps.tile([128, 1], f32)
        gam_ps = ps.tile([128, 1], f32)
        hlo_ps = ps.tile([128, M], f32)
        hhi_ps = ps.tile([128, M], f32)

        # 3D views [128, 16, 256] over the stencil operands
        def v3(ap2d):
            return ap2d.rearrange("p (c j) -> p c j", c=COLS, j=M)

        p_c2 = p[:, M:M + F]
        p_c3 = v3(p_c2)
        p_jm1 = v3(p[:, M - 1:M - 1 + F])
        p_jp1 = v3(p[:, M + 1:M + 1 + F])
        p_km1 = v3(p[:, 0:F])
        p_kp1 = v3(p[:, 2 * M:2 * M + F])
        cL3 = cL[:].rearrange("p (c j) -> p c j", c=COLS, j=M + 1)
        cLl = cL3[:, :, 0:M]        # multiplies p_jm1
        cLr = cL3[:, :, 1:M + 1]    # multiplies p_jp1 (= K_right view)
        cB3 = cB[:].rearrange("p (c j) -> p c j", c=COLS + 1, j=M)
        cBb = cB3[:, 0:COLS, :]     # multiplies p_km1
        cBt = cB3[:, 1:COLS + 1, :] # multiplies p_kp1 (= K_top view)
        cD3 = v3(cD[:])

        # ---- load inputs
        nc.sync.dma_start(cD[:], cD_d)
        nc.sync.dma_start(cL[:], cL_d)
        nc.sync.dma_start(cB[:], cB_d)
        nc.sync.dma_start(bc128[:], bc128_d)
        nc.sync.dma_start(sdn[:], sdn_d)
        nc.sync.dma_start(sup[:], sup_d)
        nc.sync.dma_start(r[:], f_d)
        nc.sync.dma_start(p_c2, f_d)

        def halo_update():
            # halo_lo[P] = center_last_col[P-1]; halo_hi[P] = center_first_col[P+1]
            nc.tensor.matmul(hlo_ps[:], sdn[:], p[:, F:F + M])
            nc.tensor.matmul(hhi_ps[:], sup[:], p[:, M:2 * M])
            nc.scalar.copy(p[:, 0:M], hlo_ps[:])
            nc.scalar.copy(p[:, F + M:F + 2 * M], hhi_ps[:])

        # ---- init: x=0, gamma0 = per-problem ||f||^2, p halos
        nc.vector.memset(x[:], 0.0)
        halo_update()
        nc.scalar.activation(t1[:], r[:], Act.Square, accum_out=rr_part[:])
        nc.tensor.matmul(gam_ps[:], bc128[:], rr_part[:])
        nc.scalar.copy(gamvec[:], gam_ps[:])

        # ---- 300 CG iterations
        loop_mode = os.environ.get("KERNEL_LOOP", "unroll4")

        def body(_i):
            # 1/gamma_old for beta, overlappable with the stencil
            nc.vector.reciprocal(recg[:], gamvec[:])

            # q = A @ p  (GpSimd: k-shift products; DVE: the rest)
            nc.gpsimd.tensor_mul(v3(t2[:]), cBb, p_km1)
            nc.gpsimd.tensor_mul(v3(t3[:]), cBt, p_kp1)
            nc.vector.tensor_mul(v3(t0[:]), cD3, p_c3)
            nc.vector.tensor_mul(v3(t1[:]), cLl, p_jm1)
            nc.vector.tensor_add(t0[:], t0[:], t1[:])
            nc.vector.tensor_mul(v3(t1[:]), cLr, p_jp1)
            nc.vector.tensor_add(t0[:], t0[:], t1[:])
            nc.vector.tensor_add(t0[:], t0[:], t2[:])
            nc.vector.tensor_add(q[:], t0[:], t3[:])

            # pAp = sum(p*q) fused in one DVE pass; alpha = gamma/pAp
            nc.vector.scalar_tensor_tensor(
                t3[:], p_c2, 1.0, q[:], Alu.mult, Alu.mult,
                accum_out=pap_part[:])
            nc.tensor.matmul(pap_ps[:], bc128[:], pap_part[:])
            nc.vector.reciprocal(recp[:], pap_ps[:])
            nc.vector.tensor_mul(avec[:], gamvec[:], recp[:])
            nc.vector.tensor_scalar_mul(aneg[:], avec[:], -1.0)

            # r = (q * -alpha) + r, one pass; x += alpha*p off-critical
            nc.vector.scalar_tensor_tensor(
                r[:], q[:], aneg[:], r[:], Alu.mult, Alu.add)
            nc.scalar.activation(t3[:], p_c2, Act.Copy, scale=avec[:])
            nc.gpsimd.tensor_add(x[:], x[:], t3[:])

            # gamma' = sum(r*r) fused on DVE (no engine hop); beta
            nc.vector.scalar_tensor_tensor(
                t2[:], r[:], 1.0, r[:], Alu.mult, Alu.mult,
                accum_out=rr_part[:])
            nc.tensor.matmul(gam_ps[:], bc128[:], rr_part[:])
            nc.vector.tensor_mul(bvec[:], gam_ps[:], recg[:])
            nc.scalar.copy(gamvec[:], gam_ps[:])

            # p = (p * beta) + r in one pass, then refresh halos
            nc.vector.scalar_tensor_tensor(
                p_c2, p_c2, bvec[:], r[:], Alu.mult, Alu.add)
            halo_update()

        if loop_mode == "plain":
            with tc.For_i(0, iters) as _i:
                body(_i)
        elif loop_mode == "stag":
            with tc.For_i(0, iters, staggered_reset=True) as _i:
                body(_i)
        elif loop_mode.startswith("unroll"):
            tc.For_i_unrolled(0, iters, 1, body, max_unroll=int(loop_mode[6:]))
        else:
            raise ValueError(loop_mode)

        nc.sync.dma_start(x_d, x[:])

    nc.compile()
    return nc


VARIANT = os.environ.get("KERNEL_VARIANT", "v3")


def _get_nc(iters, variant=None):
    variant = variant or VARIANT
    key = ("nc", iters, variant, os.environ.get("KERNEL_LOOP", "unroll4"),
           os.environ.get("KERNEL_SPLITS", ""), os.environ.get("KERNEL_KPROD", ""),
           os.environ.get("KERNEL_XUPD", ""), os.environ.get("KERNEL_RR", ""))
    if key not in _CACHE:
        builder = {"std": _build_nc, "qrec": _build_nc_qrec, "v2": _build_nc_v2,
                   "v3": _build_nc_v3}[variant]
        _CACHE[key] = builder(iters)
    return _CACHE[key]


def _expected_inputs(nc):
    import concourse.mybir as mybir
    part = nc.partition_id_tensor.name if nc.partition_id_tensor else None
    names = set()
    for alloc in nc.m.functions[0].allocations:
        if isinstance(alloc, mybir.MemoryLocationSet) and alloc.kind == "ExternalInput":
            nm = alloc.memorylocations[0].name
            if nm != part:
                names.add(nm)
    return names


# ------------------------------------------------------------------- runner

def _make_runner(iters, variant=None):
    """Build the 8-core sharded jit once; returns run(in_maps) -> [x_out]*8."""
    import jax
    from jax.sharding import Mesh, PartitionSpec
    from jax.experimental.shard_map import shard_map
    from concourse import bass2jax, mybir

    nc = _get_nc(iters, variant)
    bass2jax.install_neuronx_cc_hook()
    partition_name = nc.partition_id_tensor.name if nc.partition_id_tensor else None
    in_names, out_names, out_avals, zero_outs = [], [], [], []
    for alloc in nc.m.functions[0].allocations:
        if not isinstance(alloc, mybir.MemoryLocationSet):
            continue
        name = alloc.memorylocations[0].name
        if alloc.kind == "ExternalInput":
            if name != partition_name:
                in_names.append(name)
        elif alloc.kind == "ExternalOutput":
            out_names.append(name)
            shape = tuple(alloc.tensor_shape)
            dtype = mybir.dt.np(alloc.dtype)
            out_avals.append(jax.core.ShapedArray(shape, dtype))
            zero_outs.append(np.zeros(shape, dtype))
    n_params = len(in_names)
    all_in = in_names + out_names + ([partition_name] if partition_name else [])

    def _body(*args):
        ops = list(args)
        if partition_name:
            ops.append(bass2jax.partition_id_tensor())
        return tuple(bass2jax._bass_exec_p.bind(
            *ops, out_avals=tuple(out_avals), in_names=tuple(all_in),
            out_names=tuple(out_names), lowering_input_output_aliases=(),
            sim_require_finite=True, sim_require_nnan=True, nc=nc))

    mesh = Mesh(np.asarray(jax.devices()[:NCORES]), ("core",))
    jf = jax.jit(
        shard_map(_body, mesh=mesh,
                  in_specs=(PartitionSpec("core"),) * (n_params + len(out_names)),
                  out_specs=(PartitionSpec("core"),) * len(out_names),
                  check_rep=False),
        donate_argnums=tuple(range(n_params, n_params + len(out_names))),
        keep_unused=True)

    def prepare(in_maps):
        import jax
        concat_in = [np.concatenate([m[nm] for m in in_maps], axis=0)
                     for nm in in_names]
        dev_in = [jax.device_put(a) for a in concat_in]
        jax.block_until_ready(dev_in)
        return dev_in

    def run_dev(dev_in, fetch=True):
        import jax
        zeros = [np.zeros((NCORES * z.shape[0], *z.shape[1:]), z.dtype)
                 for z in zero_outs]
        outs = jf(*dev_in, *zeros)
        if not fetch:
            jax.block_until_ready(outs)
            return None
        xo = np.asarray(outs[0])
        per_core_rows = xo.shape[0] // NCORES
        return [xo[c * per_core_rows:(c + 1) * per_core_rows] for c in range(NCORES)]

    def run(in_maps):
        return run_dev(prepare(in_maps))

    run.prepare = prepare
    run.run_dev = run_dev
    return run


def _get_runner(iters, variant=None):
    variant = variant or VARIANT
    key = ("runner", iters, variant, os.environ.get("KERNEL_LOOP", "unroll4"),
           os.environ.get("KERNEL_SPLITS", ""), os.environ.get("KERNEL_KPROD", ""),
           os.environ.get("KERNEL_XUPD", ""), os.environ.get("KERNEL_RR", ""))
    if key not in _CACHE:
        _CACHE[key] = _make_runner(iters, variant)
    return _CACHE[key]


def _run(in_maps, iters, variant=None):
    return _get_runner(iters, variant)(in_maps)


def kernel(alpha, f_rhs):
    alpha = np.asarray(alpha, np.float32)
    f_rhs = np.asarray(f_rhs, np.float32)
    in_maps = [_pack_core(alpha[c * BPC:(c + 1) * BPC], f_rhs)
               for c in range(NCORES)]
    try:
        outs = _run(in_maps, ITERS)
    except Exception:
        # a crashed prior session can leave a core wedged; one retry clears it
        outs = _run(in_maps, ITERS)
    unpack = _from_dev_pad if VARIANT == "v3" else _from_dev
    return np.concatenate([unpack(o) for o in outs], axis=0)

